# revision 87
# baseline (speedup 1.0000x reference)
"""Trainium2 Bass kernel for nn_Brain_connectomic_graph (GNN message passing).

Single tiny graph (N=100 nodes, E=2000 edges); whole network as dense linear
algebra on ONE NeuronCore, replicated across 8 cores (data-parallel lanes,
batch=1 per the sharding hint); core 0's output is returned.

v3 design (latency-focused):
  - Adjacency densification done on the HOST as pure data placement: edges
    scattered into K=3 duplicate-slab grids (a duplicate (src,dst) pair goes
    to the next slab; no host arithmetic). Device sums slabs with 2 adds.
  - No unweighted grid: A1 (counts) comes from binarizing the weighted grid
    on GpSimd (all edge weights are nonzero).
  - No grid diagonal: the GCN +1 self-loop degree enters via the Sqrt
    activation's free bias; the +I adjacency term via one add with the
    on-device identity.
  - Degrees come from the dst-major grid via free-axis reduces (V only).
  - GCN layers alternate node-major/feature-major layouts -> NO transposes
    between layers; hemisphere selection via host-masked X^T stationaries
    (layer 1) and a 2-op DVE select (layer 2).
  - Layer biases enter as EXTRA CONTRACTION ROWS: stationaries/movings are
    augmented to k=101/102 with [bias rows | hemisphere masks], so bias
    needs no separate matmul or vector op anywhere.
  - dis sandwich built once per adjacency (shared by both layers).
  - ChebConv reassociated: s_raw = h2@(Wc0-Wc2) + G@(h2@Wc1) + 2G@(G@(h2@Wc2))
    with G the sandwiched pooled adjacency -- no Tx transposes.
  - Pooled-degree rsqrt via integer one-hot lookup against a host 1/sqrt(k)
    table (2 DVE ops, no Scalar Sqrt mid-kernel).
  - Scalar ACT tables: Sqrt set prewarmed during DMA; Exp/Tanh set loaded
    right after the single early Sqrt -- no stalls later.
"""

import numpy as np

N = 100
E = 2000
K1 = 50
KSLOT = 3          # duplicate-edge slabs (max multiplicity in data is 3)
GC = KSLOT * 100   # grid columns

# ---- inbuf column layout (f32 blob [128, C]) --------------------------------
_off = 0
def _nxt(w):
    global _off
    o = _off
    _off += w
    return o

# DMA group D (first: gates the degree/dis chain)
O_GWD  = _nxt(GC)     # [100,3,100] GWd[d, k, s] = ew(s->d), no diag
O_MBD  = _nxt(100)    # [100,100] same-hemisphere block mask
C_DMA_D = _off
# DMA group A
O_GW   = _nxt(GC)     # [100,3,100] GW[s, k, d] = ew(s->d), no diag
C_DMA_A = _off
# DMA group B: first-matmul operands
O_XTL  = _nxt(100)    # [100,100] x^T with cols (nodes) >=50 zeroed
O_XTR  = _nxt(100)    # [100,100] x^T with cols (nodes) <50 zeroed
O_W1   = _nxt(128)    # [100,128] [Wl1 | Wr1]
C_DMA_B = _off
# DMA group C: small weights/tables
O_MKL  = _nxt(1)      # [100,1] 1.0 for p<50
O_MKR  = _nxt(1)      # [100,1] 1.0 for 50<=p<100
O_BREL = _nxt(1)      # [128,1] brel broadcast
O_W2   = _nxt(40)     # [64,40] [Wl2|Wr2]
O_RSQ  = _nxt(64)     # [50,64] 1/sqrt(k) lookup rows
O_WG   = _nxt(20)     # [20,20] Wg1
O_WC0  = _nxt(20)     # [20,20] Wc0
O_WCC  = _nxt(40)     # [20,40] [Wc1 | Wc2]
O_WRR  = _nxt(2)      # [20,2]  [Wrel | Wroot]
C_COLS = _off
NRSQ = 64

# AUG mega-tile column layout: all bias-augmented stationaries/movings live in
# one [128, CAUG_COLS] tile so a single DMA fills every aug row (rows 100:102)
A_Y1, A_Y2, A_YG, A_ACT, A_AGT, A_WC0, A_H2T = 0, 64, 84, 104, 204, 304, 324
CAUG_COLS = 424


def _split_multiwaits(bir: dict) -> dict:
    """This container's walrus accepts only ONE sync-wait per instruction.
    Insert single-wait NoOps (same engine, just before) for the extras."""
    for f in bir.get("functions", []):
        for bb in f.get("blocks", []):
            out = []
            for ins in bb.get("instructions", []):
                si = ins.get("sync_info")
                waits = (si or {}).get("on_wait") or []
                if len(waits) > 1:
                    for i, w in enumerate(waits[:-1]):
                        out.append({
                            "debug": ins.get("debug", 0),
                            "engine": ins["engine"],
                            "ins": [], "outs": [],
                            "name": f"{ins['name']}-w{i}",
                            "opcode": "NoOp",
                            "sync_info": {"on_wait": [w], "on_update": []},
                        })
                    si["on_wait"] = [waits[-1]]
                out.append(ins)
            bb["instructions"] = out
    return bir


def _build():
    import concourse.bass as bass
    import concourse.mybir as mybir
    import concourse.tile as tile

    f32 = mybir.dt.float32
    Alu = mybir.AluOpType
    Act = mybir.ActivationFunctionType
    AxX = mybir.AxisListType.X

    nc = bass.Bass("TRN2")
    in_d = nc.dram_tensor("inbufD", [128, C_DMA_D], f32, kind="ExternalInput")
    in_a = nc.dram_tensor("inbufA", [128, C_DMA_A - C_DMA_D], f32, kind="ExternalInput")
    in_b = nc.dram_tensor("inbufB", [128, C_DMA_B - C_DMA_A], f32, kind="ExternalInput")
    in_c = nc.dram_tensor("inbufC", [128, C_COLS - C_DMA_B], f32, kind="ExternalInput")
    in_e = nc.dram_tensor("inbufE", [2, CAUG_COLS], f32, kind="ExternalInput")
    out_d = nc.dram_tensor("out", [K1, 20], f32, kind="ExternalOutput")

    with tile.TileContext(nc) as tc:
        with (
            tc.tile_pool(name="sb", bufs=1) as sb,
            tc.tile_pool(name="ps", bufs=1, space="PSUM") as ps,
        ):
            ibD = sb.tile([128, C_DMA_D], f32, tag="ibD", name="ibD")
            nc.sync.dma_start(out=ibD[:, :], in_=in_d.ap())
            ibA = sb.tile([128, C_DMA_A - C_DMA_D], f32, tag="ibA", name="ibA")
            nc.sync.dma_start(out=ibA[:, :], in_=in_a.ap())
            ibB = sb.tile([128, C_DMA_B - C_DMA_A], f32, tag="ibB", name="ibB")
            nc.sync.dma_start(out=ibB[:, :], in_=in_b.ap())
            ibC = sb.tile([128, C_COLS - C_DMA_B], f32, tag="ibC", name="ibC")
            nc.sync.dma_start(out=ibC[:, :], in_=in_c.ap())

            GWD  = ibD[0:100, O_GWD:O_GWD + GC]
            MBD  = ibD[0:100, O_MBD:O_MBD + 100]
            GW   = ibA[0:100, 0:GC]
            XTL  = ibB[0:100, O_XTL - C_DMA_A:O_XTL - C_DMA_A + 100]
            XTR  = ibB[0:100, O_XTR - C_DMA_A:O_XTR - C_DMA_A + 100]
            W1   = ibB[0:100, O_W1 - C_DMA_A:O_W1 - C_DMA_A + 128]
            def icl(off, w, p0=0, p1=128):
                return ibC[p0:p1, off - C_DMA_B:off - C_DMA_B + w]
            MKL  = icl(O_MKL, 1, 0, 100)
            MKR  = icl(O_MKR, 1, 0, 100)
            BREL = icl(O_BREL, 1)
            W2   = icl(O_W2, 40, 0, 64)
            RSQ  = icl(O_RSQ, NRSQ, 0, 50)
            WG   = icl(O_WG, 20, 0, 20)
            WC0  = icl(O_WC0, 20, 0, 20)
            WCC  = icl(O_WCC, 40, 0, 20)
            WRR2 = icl(O_WRR, 2, 0, 20)

            V = nc.vector
            S = nc.scalar
            P = nc.gpsimd
            T = nc.tensor
            mm = lambda shape, name: ps.tile(shape, f32, tag="mm", name=name, bufs=7)

            # AUG mega-tile: all bias-augmented operands; zeroed once, aug rows
            # (100:102) filled by ONE small DMA
            AUG = sb.tile([128, CAUG_COLS], f32, tag="AUG", name="AUG")
            V.memset(AUG, 0.0)
            nc.sync.dma_start(out=AUG[100:102, :], in_=in_e.ap())
            y1aug = AUG[0:102, A_Y1:A_Y1 + 64]
            y2aug = AUG[0:102, A_Y2:A_Y2 + 20]
            ygaug = AUG[0:101, A_YG:A_YG + 20]
            acts_aug = AUG[0:102, A_ACT:A_ACT + 100]
            agts_aug = AUG[0:101, A_AGT:A_AGT + 100]
            wc0paug = AUG[0:102, A_WC0:A_WC0 + 20]
            h2Taug = AUG[0:102, A_H2T:A_H2T + 100]
            act_s = AUG[0:100, A_ACT:A_ACT + 100]
            agt_s = AUG[0:100, A_AGT:A_AGT + 100]
            h2T = AUG[0:20, A_H2T:A_H2T + 100]
            wc0p = AUG[0:20, A_WC0:A_WC0 + 20]

            # ---- prologue: ACT sqrt-set prewarm + PE warmup (HAM ramp) ------
            scr = sb.tile([1, 1], f32, tag="scr", name="scr")
            V.memset(scr, 0.0)
            S.activation(out=scr, in_=scr, func=Act.Sqrt)
            wmt = sb.tile([128, 100], f32, tag="wmt", name="wmt")
            V.memset(wmt, 1.0)
            warm = ps.tile([100, 200], f32, tag="warm", name="warm", bufs=1)
            wm_b = wmt.unsqueeze(1).broadcast_to([128, 2, 100])
            for _ in range(4):
                T.matmul(warm, wmt, wm_b)

            # ---- on-device constants (GpSimd, runs during the DMAs) ---------
            iota_i = sb.tile([128, 100], mybir.dt.int32, tag="iota_i", name="iota_i")
            P.iota(iota_i, pattern=[[1, 100]], base=0, channel_multiplier=0)
            iota_t = sb.tile([128, 100], f32, tag="iota_t", name="iota_t")
            P.tensor_copy(out=iota_t, in_=iota_i)
            i100_t = sb.tile([100, 100], f32, tag="i100_t", name="i100_t")
            P.memset(i100_t, 0.0)
            P.affine_select(out=i100_t, in_=i100_t, compare_op=Alu.not_equal,
                            fill=1.0, base=0, pattern=[[-1, 100]], channel_multiplier=1)
            tril_t = sb.tile([100, 100], f32, tag="tril_t", name="tril_t")
            P.memset(tril_t, 1.0)
            P.affine_select(out=tril_t, in_=tril_t, compare_op=Alu.is_gt,
                            fill=0.0, base=0, pattern=[[-1, 100]], channel_multiplier=1)
            triu_t = sb.tile([100, 100], f32, tag="triu_t", name="triu_t")
            P.memset(triu_t, 1.0)
            P.affine_select(out=triu_t, in_=triu_t, compare_op=Alu.is_gt,
                            fill=0.0, base=0, pattern=[[1, 100]], channel_multiplier=-1)
            ones_t = sb.tile([1, 100], f32, tag="ones_t", name="ones_t")
            P.memset(ones_t, 1.0)
            ONESR = ones_t[0:1, :]
            onesq = sb.tile([100, 100], f32, tag="onesq", name="onesq")
            P.memset(onesq, 1.0)
            I100 = i100_t[:, :]
            I20 = i100_t[0:20, 0:20]
            I50 = i100_t[0:50, 0:50]
            IO50 = iota_t[0:100, 0:50]
            IO64 = iota_t[0:50, 0:NRSQ]
            TRIL = tril_t[:, :]
            TRIU = triu_t[:, :]

            # ---- degrees straight off the dst-major grid --------------------
            # GWd's columns are host-permuted so same-hemisphere sources sit in
            # cols 0:50 of every slab -> deg_c is a plain subrange reduce
            dd = sb.tile([100, 2], f32, tag="dd", name="dd")
            gwd3 = GWD.rearrange("p (c j) -> p c j", c=KSLOT)
            V.tensor_reduce(out=dd[:, 0:1], in_=gwd3[:, :, 0:50],
                            axis=mybir.AxisListType.XY, op=Alu.add)
            V.tensor_reduce(out=dd[:, 1:2], in_=gwd3, axis=mybir.AxisListType.XY, op=Alu.add)
            # dis = 1/sqrt(deg+1): +1 self-loop via Sqrt's free bias
            sq2 = sb.tile([100, 2], f32, tag="sq2", name="sq2")
            S.activation(out=sq2, in_=dd, func=Act.Sqrt, bias=1.0)
            # switch Scalar ACT table to the Exp/Tanh set right after the last
            # Sqrt (input dep on sq2 pins the order; the load then overlaps
            # the GCN layers instead of stalling the tail)
            S.activation(out=scr, in_=sq2[0:1, 0:1], func=Act.Tanh)
            rdis = sb.tile([100, 2], f32, tag="rdis", name="rdis")
            V.reciprocal(out=rdis, in_=sq2)
            rdis_c = rdis[:, 0:1]
            rdis_g = rdis[:, 1:2]

            # ---- adjacency slab sums (adds on GpSimd, compare on DVE) ------
            agtmp = sb.tile([100, 100], f32, tag="agtmp", name="agtmp")
            P.tensor_tensor(out=agtmp, in0=GW[:, 0:100], in1=GW[:, 100:200], op=Alu.add)
            agts = sb.tile([100, 100], f32, tag="agts", name="agts")
            P.tensor_tensor(out=agts, in0=agtmp, in1=GW[:, 200:300], op=Alu.add)
            agt = sb.tile([100, 100], f32, tag="agt", name="agt")
            P.tensor_tensor(out=agt, in0=agts, in1=I100, op=Alu.add)
            act = sb.tile([100, 100], f32, tag="act", name="act")
            P.tensor_tensor(out=act, in0=agt, in1=MBD, op=Alu.mult)
            # ---- dis sandwich for both adjacencies --------------------------
            # row-replicate dis via ones @ diag(dis); diag built on idle GpSimd
            dgc = sb.tile([100, 100], f32, tag="dgc", name="dgc")
            P.affine_select(out=dgc, in_=rdis_c.broadcast_to([100, 100]),
                            compare_op=Alu.is_equal, fill=0.0, base=0,
                            pattern=[[-1, 100]], channel_multiplier=1)
            drep_c = mm([100, 100], "drep_c")
            T.matmul(drep_c, onesq, dgc)
            V.scalar_tensor_tensor(out=act_s, in0=drep_c, scalar=rdis_c, in1=act,
                                   op0=Alu.mult, op1=Alu.mult)

            # Wc0' = Wc0 - Wc2 (early, off critical path)
            V.tensor_tensor(out=wc0p, in0=WC0, in1=WCC[:, 20:40], op=Alu.subtract)

            # unweighted counts (all ew > 0); emitted after the deg/sandwich
            # chain so the scheduler doesn't slot them ahead of it
            b3 = sb.tile([100, GC], f32, tag="b3", name="b3")
            V.tensor_scalar(out=b3, in0=GW, scalar1=0.0, scalar2=None, op0=Alu.is_gt)
            a1tmp = sb.tile([100, 100], f32, tag="a1tmp", name="a1tmp")
            V.tensor_tensor(out=a1tmp, in0=b3[:, 0:100], in1=b3[:, 100:200], op=Alu.add)
            a1t = sb.tile([100, 100], f32, tag="a1t", name="a1t")
            V.tensor_tensor(out=a1t, in0=a1tmp, in1=b3[:, 200:300], op=Alu.add)

            # ---- layer 1 (out feature-major [64,100]) -----------------------
            xw_ps = mm([100, 64], "xw_ps")
            T.matmul(xw_ps, XTL, W1[:, 0:64], start=True, stop=False)
            T.matmul(xw_ps, XTR, W1[:, 64:128], start=False, stop=True)
            V.tensor_copy(out=AUG[0:100, A_Y1:A_Y1 + 64], in_=xw_ps)
            z1T = mm([64, 100], "z1T")
            T.matmul(z1T, y1aug, acts_aug)
            # global-layer sandwich off the critical path (first use ~5us out)
            dgg = sb.tile([100, 100], f32, tag="dgg", name="dgg")
            P.affine_select(out=dgg, in_=rdis_g.broadcast_to([100, 100]),
                            compare_op=Alu.is_equal, fill=0.0, base=0,
                            pattern=[[-1, 100]], channel_multiplier=1)
            drep_g = mm([100, 100], "drep_g")
            T.matmul(drep_g, onesq, dgg)
            V.scalar_tensor_tensor(out=agt_s, in0=drep_g, scalar=rdis_g, in1=agt,
                                   op0=Alu.mult, op1=Alu.mult)
            z1s = sb.tile([64, 100], f32, tag="z1s", name="z1s")
            V.tensor_copy(out=z1s, in_=z1T)
            h1t = sb.tile([64, 100], f32, tag="h1t", name="h1t")
            V.scalar_tensor_tensor(out=h1t, in0=z1s, scalar=0.01, in1=z1s,
                                   op0=Alu.mult, op1=Alu.max)

            # ---- layer 2 ----------------------------------------------------
            xw2 = mm([100, 40], "xw2")
            T.matmul(xw2, h1t, W2)
            y2r = sb.tile([100, 20], f32, tag="y2r", name="y2r")
            V.tensor_scalar_mul(y2r, xw2[:, 20:40], MKR)
            V.scalar_tensor_tensor(out=AUG[0:100, A_Y2:A_Y2 + 20], in0=xw2[:, 0:20],
                                   scalar=MKL, in1=y2r, op0=Alu.mult, op1=Alu.add)
            z2T = mm([20, 100], "z2T")
            T.matmul(z2T, y2aug, acts_aug)
            z2s = sb.tile([20, 100], f32, tag="z2s", name="z2s")
            V.tensor_copy(out=z2s, in_=z2T)
            h2at = sb.tile([20, 100], f32, tag="h2at", name="h2at")
            V.scalar_tensor_tensor(out=h2at, in0=z2s, scalar=0.01, in1=z2s,
                                   op0=Alu.mult, op1=Alu.max)

            # ---- global GCN layer ------------------------------------------
            xwg = mm([100, 20], "xwg")
            T.matmul(xwg, h2at, WG)
            V.tensor_copy(out=AUG[0:100, A_YG:A_YG + 20], in_=xwg)
            zgT = mm([20, 100], "zgT")
            T.matmul(zgT, ygaug, agts_aug)
            zgs = sb.tile([20, 100], f32, tag="zgs", name="zgs")
            V.tensor_copy(out=zgs, in_=zgT)
            V.scalar_tensor_tensor(out=h2T, in0=zgs, scalar=0.01, in1=zgs,
                                   op0=Alu.mult, op1=Alu.max)

            # ---- SAGPool score (critical: emitted before h2x/Cheb mms) ------
            h2x = sb.tile([100, 21], f32, tag="h2x", name="h2x")
            score = h2x[:, 20:21]
            hw_ps = mm([100, 2], "hw_ps")
            T.matmul(hw_ps, h2T, WRR2)
            hw = sb.tile([100, 2], f32, tag="hw", name="hw")
            V.tensor_copy(out=hw, in_=hw_ps)
            sc_ps = mm([100, 1], "sc_ps")
            T.matmul(sc_ps, a1t, hw[:, 0:1])
            V.tensor_tensor(out=score, in0=sc_ps, in1=hw[:, 1:2], op=Alu.add)

            # h2 node-major into h2x cols 0:20; Cheb products (need only h2T)
            h2x_p = mm([100, 20], "h2x_p")
            T.transpose(h2x_p, h2T, I20)
            V.tensor_copy(out=h2x[:, 0:20], in_=h2x_p)
            pp_ps = mm([100, 40], "pp_ps")
            T.matmul(pp_ps, h2T, WCC)
            pp = sb.tile([50, 40], f32, tag="pp", name="pp")
            V.tensor_copy(out=pp, in_=pp_ps[0:50, :])
            sraw_ps = mm([100, 20], "sraw_ps")
            T.matmul(sraw_ps, h2Taug, wc0paug, start=True, stop=False)

            # ---- rank / top-k ----------------------------------------------
            dgs = sb.tile([100, 100], f32, tag="dgs", name="dgs")
            P.affine_select(out=dgs, in_=score.broadcast_to([100, 100]),
                            compare_op=Alu.is_equal, fill=0.0, base=0,
                            pattern=[[-1, 100]], channel_multiplier=1)
            srep_ps = mm([100, 100], "srep_ps")
            T.matmul(srep_ps, onesq, dgs)
            t2 = sb.tile([100, 100], f32, tag="t2", name="t2")
            V.scalar_tensor_tensor(out=t2, in0=srep_ps, scalar=score, in1=TRIL,
                                   op0=Alu.is_equal, op1=Alu.mult)
            csum = sb.tile([100, 100], f32, tag="csum", name="csum")
            rank = sb.tile([100, 1], f32, tag="rank", name="rank")
            V.scalar_tensor_tensor(out=csum, in0=srep_ps, scalar=score, in1=t2,
                                   op0=Alu.is_gt, op1=Alu.add, accum_out=rank)
            kept = sb.tile([100, 1], f32, tag="kept", name="kept")
            V.tensor_scalar(out=kept, in0=rank, scalar1=49.5, scalar2=None, op0=Alu.is_lt)
            pit = sb.tile([100, 50], f32, tag="pit", name="pit")
            V.tensor_scalar(out=pit, in0=IO50, scalar1=rank, scalar2=None, op0=Alu.is_equal)

            # ---- pooled rows + gather matrix -------------------------------
            w_ps = mm([100, 1], "w_ps")
            T.matmul(w_ps, a1t, kept)
            w_sb = sb.tile([100, 1], f32, tag="w_sb", name="w_sb")
            V.tensor_copy(out=w_sb, in_=w_ps)
            m1 = mm([100, 50], "m1")
            T.matmul(m1, a1t, pit)
            m1s = sb.tile([100, 50], f32, tag="m1s", name="m1s")
            V.tensor_copy(out=m1s, in_=m1)
            degc_p = mm([50, 1], "degc_p")
            T.matmul(degc_p, pit, w_sb)
            atilt_p = mm([50, 50], "atilt_p")
            T.matmul(atilt_p, m1s, pit)
            p1 = mm([50, 21], "p1")
            T.matmul(p1, pit, h2x)
            th = sb.tile([50, 1], f32, tag="th", name="th")
            S.activation(out=th, in_=p1[:, 20:21], func=Act.Tanh, bias=BREL[0:50, :], scale=1.0)
            p1s = sb.tile([50, 20], f32, tag="p1s", name="p1s")
            V.tensor_copy(out=p1s, in_=p1[:, 0:20])
            srank_p = mm([100, 1], "srank_p")
            T.matmul(srank_p, TRIU, kept)
            gat = sb.tile([100, 50], f32, tag="gat", name="gat")
            V.scalar_tensor_tensor(out=gat, in0=IO50, scalar=srank_p,
                                   in1=kept.broadcast_to([100, 50]),
                                   op0=Alu.is_equal, op1=Alu.mult)

            # pooled-degree rsqrt via integer one-hot lookup (no Scalar Sqrt)
            ohscr = sb.tile([50, NRSQ], f32, tag="ohscr", name="ohscr")
            V.scalar_tensor_tensor(out=ohscr, in0=IO64, scalar=degc_p, in1=RSQ,
                                   op0=Alu.is_equal, op1=Alu.mult)
            disch = sb.tile([50, 1], f32, tag="disch", name="disch")
            V.tensor_reduce(out=disch, in_=ohscr, axis=AxX, op=Alu.add)
            ndisch = sb.tile([50, 1], f32, tag="ndisch", name="ndisch")
            V.tensor_scalar_mul(ndisch, disch, -1.0)
            dgd = sb.tile([50, 50], f32, tag="dgd", name="dgd")
            P.affine_select(out=dgd, in_=disch.broadcast_to([50, 50]),
                            compare_op=Alu.is_equal, fill=0.0, base=0,
                            pattern=[[-1, 50]], channel_multiplier=1)
            drepd = mm([50, 50], "drepd")
            T.matmul(drepd, onesq[0:50, 0:50], dgd)
            gsx1 = sb.tile([50, 50], f32, tag="gsx1", name="gsx1")
            V.tensor_scalar_mul(gsx1, atilt_p, ndisch)
            gsx = sb.tile([50, 100], f32, tag="gsx", name="gsx")
            V.memset(gsx, 0.0)
            V.tensor_tensor(out=gsx[:, 0:50], in0=drepd, in1=gsx1, op=Alu.mult)

            # ---- Cheb accumulation into sraw -------------------------------
            T.matmul(sraw_ps, gsx, pp[:, 0:20], start=False, stop=False)
            q2_ps = mm([100, 20], "q2_ps")
            T.matmul(q2_ps, gsx, pp[:, 20:40])
            q2x2 = sb.tile([50, 20], f32, tag="q2x2", name="q2x2")
            V.tensor_scalar_mul(q2x2, q2_ps[0:50, :], 2.0)
            T.matmul(sraw_ps, gsx, q2x2, start=False, stop=True)

            # ---- double softmax (normalizations folded) --------------------
            ex1 = sb.tile([100, 20], f32, tag="ex1", name="ex1")
            sum1 = sb.tile([100, 1], f32, tag="sum1", name="sum1")
            S.activation(out=ex1, in_=sraw_ps, func=Act.Exp, accum_out=sum1)
            rc1 = sb.tile([100, 1], f32, tag="rc1", name="rc1")
            V.reciprocal(out=rc1, in_=sum1)
            exr = sb.tile([100, 20], f32, tag="exr", name="exr")
            V.tensor_scalar_mul(exr, ex1, rc1)
            ex2 = sb.tile([100, 20], f32, tag="ex2", name="ex2")
            sum2 = sb.tile([100, 1], f32, tag="sum2", name="sum2")
            S.activation(out=ex2, in_=ex1, func=Act.Exp, scale=rc1, accum_out=sum2)
            rc2 = sb.tile([100, 1], f32, tag="rc2", name="rc2")
            V.reciprocal(out=rc2, in_=sum2)
            s2 = sb.tile([100, 20], f32, tag="s2", name="s2")
            V.tensor_scalar_mul(s2, ex2, rc2)

            # ---- diff-pool tail --------------------------------------------
            # M = gat^T @ ass (runs while softmax-2 is still on Scalar)
            m_ps = mm([50, 20], "m_ps")
            T.matmul(m_ps, gat, exr)
            m_sb = sb.tile([50, 20], f32, tag="m_sb", name="m_sb")
            V.tensor_copy(out=m_sb, in_=m_ps)
            mt_ps = mm([20, 50], "mt_ps")
            T.transpose(mt_ps, m_sb, I50)
            mt = sb.tile([20, 50], f32, tag="mt", name="mt")
            V.tensor_copy(out=mt, in_=mt_ps)
            hc_ps = mm([20, 20], "hc_ps")
            T.matmul(hc_ps, s2, h2x[:, 0:20])
            hc = sb.tile([20, 20], f32, tag="hc", name="hc")
            V.tensor_copy(out=hc, in_=hc_ps)
            g_p = mm([50, 20], "g_p")
            T.matmul(g_p, mt, hc)
            outv = sb.tile([50, 20], f32, tag="outv", name="outv")
            V.scalar_tensor_tensor(out=outv, in0=p1s, scalar=th, in1=g_p,
                                   op0=Alu.mult, op1=Alu.add)
            nc.sync.dma_start(out=out_d.ap(), in_=outv, single_packet=True)

    # walrus single-wait workaround
    orig = nc.to_json_bytes
    def patched(*a, **k):
        import json as _json
        return _json.dumps(_split_multiwaits(_json.loads(orig(*a, **k)))).encode()
    nc.to_json_bytes = patched
    return nc


def _pack(inputs) -> np.ndarray:
    f = lambda k: np.asarray(inputs[k], dtype=np.float32)
    blob = np.zeros((128, C_COLS), dtype=np.float32)

    ei = np.asarray(inputs["edge_index"]).astype(np.int64)
    src, dst = ei[0], ei[1]
    ew = f("edge_attr")
    assert (ew > 0).all(), "zero edge weight breaks grid binarization"
    # scatter edges into duplicate slabs (pure placement; no arithmetic)
    slot = {}
    gwd = np.zeros((100, KSLOT, 100), np.float32)
    gw = np.zeros((100, KSLOT, 100), np.float32)
    for e in range(E):
        s, d = int(src[e]), int(dst[e])
        k = slot.get((s, d), 0)
        slot[(s, d)] = k + 1
        assert k < KSLOT, "duplicate-edge multiplicity exceeds KSLOT"
        # dst-major grid: per-row column rotation puts same-hemisphere
        # sources in cols 0:50 (pure placement)
        sc = s if d < 50 else (s + 50) % 100
        gwd[d, k, sc] = ew[e]
        gw[s, k, d] = ew[e]
    blob[0:100, O_GWD:O_GWD + GC] = gwd.reshape(100, GC)
    blob[0:100, O_GW:O_GW + GC] = gw.reshape(100, GC)

    half = np.arange(100) < 50
    blob[0:100, O_MBD:O_MBD + 100] = (half[:, None] == half[None, :]).astype(np.float32)

    x = f("x")
    xt = x.T.copy()
    xtl = xt.copy(); xtl[:, 50:] = 0.0
    xtr = xt.copy(); xtr[:, :50] = 0.0
    blob[0:100, O_XTL:O_XTL + 100] = xtl
    blob[0:100, O_XTR:O_XTR + 100] = xtr
    blob[0:100, O_W1:O_W1 + 64] = f("Wl1")
    blob[0:100, O_W1 + 64:O_W1 + 128] = f("Wr1")

    blob[0:50, O_MKL] = 1.0
    blob[50:100, O_MKR] = 1.0
    blob[:, O_BREL] = f("brel")[0]
    blob[0:64, O_W2:O_W2 + 20] = f("Wl2")
    blob[0:64, O_W2 + 20:O_W2 + 40] = f("Wr2")
    # 1/sqrt(k) lookup rows (constants; row-replicated for the free-dim dot)
    ks = np.arange(NRSQ, dtype=np.float32)
    rsq = np.zeros(NRSQ, np.float32)
    rsq[1:] = 1.0 / np.sqrt(ks[1:])
    blob[0:50, O_RSQ:O_RSQ + NRSQ] = rsq[None, :]
    blob[0:20, O_WG:O_WG + 20] = f("Wg1")
    blob[0:20, O_WC0:O_WC0 + 20] = f("Wc0")
    blob[0:20, O_WCC:O_WCC + 20] = f("Wc1")
    blob[0:20, O_WCC + 20:O_WCC + 40] = f("Wc2")
    blob[0:20, O_WRR] = f("Wrel")[:, 0]
    blob[0:20, O_WRR + 1] = f("Wroot")[:, 0]
    return blob


def _pack_aug(inputs) -> np.ndarray:
    f = lambda k: np.asarray(inputs[k], dtype=np.float32)
    aug = np.zeros((2, CAUG_COLS), np.float32)
    half = (np.arange(100) < 50).astype(np.float32)
    aug[0, A_Y1:A_Y1 + 64] = f("bl1")
    aug[1, A_Y1:A_Y1 + 64] = f("br1")
    aug[0, A_Y2:A_Y2 + 20] = f("bl2")
    aug[1, A_Y2:A_Y2 + 20] = f("br2")
    aug[0, A_YG:A_YG + 20] = f("bg1")
    aug[0, A_ACT:A_ACT + 100] = half
    aug[1, A_ACT:A_ACT + 100] = 1.0 - half
    aug[0, A_AGT:A_AGT + 100] = 1.0
    aug[0, A_WC0:A_WC0 + 20] = f("bc")
    aug[0, A_H2T:A_H2T + 100] = 1.0
    return aug


_NC = None

def _get_nc():
    global _NC
    if _NC is None:
        _NC = _build()
    return _NC


def run(inputs, trace=False):
    from concourse.bass_utils import run_bass_kernel_spmd
    nc = _get_nc()
    blob = _pack(inputs)
    parts = {
        "inbufD": np.ascontiguousarray(blob[:, 0:C_DMA_D]),
        "inbufA": np.ascontiguousarray(blob[:, C_DMA_D:C_DMA_A]),
        "inbufB": np.ascontiguousarray(blob[:, C_DMA_A:C_DMA_B]),
        "inbufC": np.ascontiguousarray(blob[:, C_DMA_B:C_COLS]),
        "inbufE": _pack_aug(inputs),
    }
    in_maps = [dict(parts) for _ in range(8)]
    res = run_bass_kernel_spmd(nc, in_maps, list(range(8)), trace=trace)
    out = np.asarray(res.results[0]["out"], dtype=np.float32).reshape(1, K1 * 20)
    return out, res


def kernel(**inputs) -> np.ndarray:
    out, _ = run(inputs)
    return out


# revision 92
# speedup vs baseline: 1.1086x; 1.1086x over previous
"""Trainium2 Bass kernel for nn_Brain_connectomic_graph (GNN message passing).

Single tiny graph (N=100 nodes, E=2000 edges); whole network as dense linear
algebra on ONE NeuronCore, replicated across 8 cores (data-parallel lanes,
batch=1 per the sharding hint); core 0's output is returned.

v3 design (latency-focused):
  - Adjacency densification done on the HOST as pure data placement: edges
    scattered into K=3 duplicate-slab grids (a duplicate (src,dst) pair goes
    to the next slab; no host arithmetic). Device sums slabs with 2 adds.
  - No unweighted grid: A1 (counts) comes from binarizing the weighted grid
    on GpSimd (all edge weights are nonzero).
  - No grid diagonal: the GCN +1 self-loop degree enters via the Sqrt
    activation's free bias; the +I adjacency term via one add with the
    on-device identity.
  - Degrees come from the dst-major grid via free-axis reduces (V only).
  - GCN layers alternate node-major/feature-major layouts -> NO transposes
    between layers; hemisphere selection via host-masked X^T stationaries
    (layer 1) and a 2-op DVE select (layer 2).
  - Layer biases enter as EXTRA CONTRACTION ROWS: stationaries/movings are
    augmented to k=101/102 with [bias rows | hemisphere masks], so bias
    needs no separate matmul or vector op anywhere.
  - dis sandwich built once per adjacency (shared by both layers).
  - ChebConv reassociated: s_raw = h2@(Wc0-Wc2) + G@(h2@Wc1) + 2G@(G@(h2@Wc2))
    with G the sandwiched pooled adjacency -- no Tx transposes.
  - Pooled-degree rsqrt via integer one-hot lookup against a host 1/sqrt(k)
    table (2 DVE ops, no Scalar Sqrt mid-kernel).
  - Scalar ACT tables: Sqrt set prewarmed during DMA; Exp/Tanh set loaded
    right after the single early Sqrt -- no stalls later.
"""

import numpy as np

N = 100
E = 2000
K1 = 50
KSLOT = 3          # duplicate-edge slabs (max multiplicity in data is 3)
GC = KSLOT * 100   # grid columns

# ---- inbuf column layout (f32 blob [128, C]) --------------------------------
_off = 0
def _nxt(w):
    global _off
    o = _off
    _off += w
    return o

# DMA group D (first: gates the degree/dis chain)
O_GWD  = _nxt(GC)     # [100,3,100] GWd[d, k, s] = ew(s->d), no diag
O_MBD  = _nxt(100)    # [100,100] same-hemisphere block mask
C_DMA_D = _off
# DMA group A
O_GW   = _nxt(GC)     # [100,3,100] GW[s, k, d] = ew(s->d), no diag
C_DMA_A = _off
# DMA group B: first-matmul operands
O_XTL  = _nxt(100)    # [100,100] x^T with cols (nodes) >=50 zeroed
O_XTR  = _nxt(100)    # [100,100] x^T with cols (nodes) <50 zeroed
O_W1   = _nxt(128)    # [100,128] [Wl1 | Wr1]
C_DMA_B = _off
# DMA group C: small weights/tables
O_MKL  = _nxt(1)      # [100,1] 1.0 for p<50
O_MKR  = _nxt(1)      # [100,1] 1.0 for 50<=p<100
O_BREL = _nxt(1)      # [128,1] brel broadcast
O_W2   = _nxt(40)     # [64,40] [Wl2|Wr2]
O_RSQ  = _nxt(64)     # [50,64] 1/sqrt(k) lookup rows
O_WG   = _nxt(20)     # [20,20] Wg1
O_WC0  = _nxt(20)     # [20,20] Wc0
O_WCC  = _nxt(40)     # [20,40] [Wc1 | Wc2]
O_WRR  = _nxt(2)      # [20,2]  [Wrel | Wroot]
C_COLS = _off
NRSQ = 64

# AUG mega-tile column layout: all bias-augmented stationaries/movings live in
# one [128, CAUG_COLS] tile so a single DMA fills every aug row (rows 100:102)
A_Y1, A_Y2, A_YG, A_ACT, A_AGT, A_WC0, A_H2T = 0, 64, 84, 104, 204, 304, 324
CAUG_COLS = 424


def _split_multiwaits(bir: dict) -> dict:
    """This container's walrus accepts only ONE sync-wait per instruction.
    Insert single-wait NoOps (same engine, just before) for the extras."""
    for f in bir.get("functions", []):
        for bb in f.get("blocks", []):
            out = []
            for ins in bb.get("instructions", []):
                si = ins.get("sync_info")
                waits = (si or {}).get("on_wait") or []
                if len(waits) > 1:
                    for i, w in enumerate(waits[:-1]):
                        out.append({
                            "debug": ins.get("debug", 0),
                            "engine": ins["engine"],
                            "ins": [], "outs": [],
                            "name": f"{ins['name']}-w{i}",
                            "opcode": "NoOp",
                            "sync_info": {"on_wait": [w], "on_update": []},
                        })
                    si["on_wait"] = [waits[-1]]
                out.append(ins)
            bb["instructions"] = out
    return bir


def _build():
    import concourse.bass as bass
    import concourse.mybir as mybir
    import concourse.tile as tile

    f32 = mybir.dt.float32
    Alu = mybir.AluOpType
    Act = mybir.ActivationFunctionType
    AxX = mybir.AxisListType.X

    nc = bass.Bass("TRN2")
    in_d = nc.dram_tensor("inbufD", [128, C_DMA_D], f32, kind="ExternalInput")
    in_a = nc.dram_tensor("inbufA", [128, C_DMA_A - C_DMA_D], f32, kind="ExternalInput")
    in_b = nc.dram_tensor("inbufB", [128, C_DMA_B - C_DMA_A], f32, kind="ExternalInput")
    in_c = nc.dram_tensor("inbufC", [128, C_COLS - C_DMA_B], f32, kind="ExternalInput")
    in_e = nc.dram_tensor("inbufE", [2, CAUG_COLS], f32, kind="ExternalInput")
    out_d = nc.dram_tensor("out", [K1, 20], f32, kind="ExternalOutput")

    with tile.TileContext(nc) as tc:
        with (
            tc.tile_pool(name="sb", bufs=1) as sb,
            tc.tile_pool(name="ps", bufs=1, space="PSUM") as ps,
        ):
            ibD = sb.tile([128, C_DMA_D], f32, tag="ibD", name="ibD")
            nc.sync.dma_start(out=ibD[:, :], in_=in_d.ap())
            ibA = sb.tile([128, C_DMA_A - C_DMA_D], f32, tag="ibA", name="ibA")
            nc.sync.dma_start(out=ibA[:, :], in_=in_a.ap())
            ibB = sb.tile([128, C_DMA_B - C_DMA_A], f32, tag="ibB", name="ibB")
            nc.sync.dma_start(out=ibB[:, :], in_=in_b.ap())
            ibC = sb.tile([128, C_COLS - C_DMA_B], f32, tag="ibC", name="ibC")
            nc.sync.dma_start(out=ibC[:, :], in_=in_c.ap())

            GWD  = ibD[0:100, O_GWD:O_GWD + GC]
            MBD  = ibD[0:100, O_MBD:O_MBD + 100]
            GW   = ibA[0:100, 0:GC]
            XTL  = ibB[0:100, O_XTL - C_DMA_A:O_XTL - C_DMA_A + 100]
            XTR  = ibB[0:100, O_XTR - C_DMA_A:O_XTR - C_DMA_A + 100]
            W1   = ibB[0:100, O_W1 - C_DMA_A:O_W1 - C_DMA_A + 128]
            def icl(off, w, p0=0, p1=128):
                return ibC[p0:p1, off - C_DMA_B:off - C_DMA_B + w]
            MKL  = icl(O_MKL, 1, 0, 100)
            MKR  = icl(O_MKR, 1, 0, 100)
            BREL = icl(O_BREL, 1)
            W2   = icl(O_W2, 40, 0, 64)
            RSQ  = icl(O_RSQ, NRSQ, 0, 50)
            WG   = icl(O_WG, 20, 0, 20)
            WC0  = icl(O_WC0, 20, 0, 20)
            WCC  = icl(O_WCC, 40, 0, 20)
            WRR2 = icl(O_WRR, 2, 0, 20)

            V = nc.vector
            S = nc.scalar
            P = nc.gpsimd
            T = nc.tensor
            mm = lambda shape, name: ps.tile(shape, f32, tag="mm", name=name, bufs=7)

            # AUG mega-tile: all bias-augmented operands; zeroed once, aug rows
            # (100:102) filled by ONE small DMA
            AUG = sb.tile([128, CAUG_COLS], f32, tag="AUG", name="AUG")
            V.memset(AUG, 0.0)
            nc.sync.dma_start(out=AUG[100:102, :], in_=in_e.ap())
            y1aug = AUG[0:102, A_Y1:A_Y1 + 64]
            y2aug = AUG[0:102, A_Y2:A_Y2 + 20]
            ygaug = AUG[0:101, A_YG:A_YG + 20]
            acts_aug = AUG[0:102, A_ACT:A_ACT + 100]
            agts_aug = AUG[0:101, A_AGT:A_AGT + 100]
            wc0paug = AUG[0:102, A_WC0:A_WC0 + 20]
            h2Taug = AUG[0:102, A_H2T:A_H2T + 100]
            act_s = AUG[0:100, A_ACT:A_ACT + 100]
            agt_s = AUG[0:100, A_AGT:A_AGT + 100]
            h2T = AUG[0:20, A_H2T:A_H2T + 100]
            wc0p = AUG[0:20, A_WC0:A_WC0 + 20]

            # ---- prologue: ACT sqrt-set prewarm + PE warmup (HAM ramp) ------
            scr = sb.tile([1, 1], f32, tag="scr", name="scr")
            V.memset(scr, 0.0)
            S.activation(out=scr, in_=scr, func=Act.Sqrt)
            wmt = sb.tile([128, 100], f32, tag="wmt", name="wmt")
            V.memset(wmt, 1.0)
            warm = ps.tile([100, 200], f32, tag="warm", name="warm", bufs=1)
            wm_b = wmt.unsqueeze(1).broadcast_to([128, 2, 100])
            for _ in range(4):
                T.matmul(warm, wmt, wm_b)

            # ---- on-device constants (GpSimd, runs during the DMAs) ---------
            iota_i = sb.tile([128, 100], mybir.dt.int32, tag="iota_i", name="iota_i")
            P.iota(iota_i, pattern=[[1, 100]], base=0, channel_multiplier=0)
            iota_t = sb.tile([128, 100], f32, tag="iota_t", name="iota_t")
            P.tensor_copy(out=iota_t, in_=iota_i)
            i100_t = sb.tile([100, 100], f32, tag="i100_t", name="i100_t")
            P.memset(i100_t, 0.0)
            P.affine_select(out=i100_t, in_=i100_t, compare_op=Alu.not_equal,
                            fill=1.0, base=0, pattern=[[-1, 100]], channel_multiplier=1)
            tril_t = sb.tile([100, 100], f32, tag="tril_t", name="tril_t")
            P.memset(tril_t, 1.0)
            P.affine_select(out=tril_t, in_=tril_t, compare_op=Alu.is_gt,
                            fill=0.0, base=0, pattern=[[-1, 100]], channel_multiplier=1)
            triu_t = sb.tile([100, 100], f32, tag="triu_t", name="triu_t")
            P.memset(triu_t, 1.0)
            P.affine_select(out=triu_t, in_=triu_t, compare_op=Alu.is_gt,
                            fill=0.0, base=0, pattern=[[1, 100]], channel_multiplier=-1)
            ones_t = sb.tile([1, 100], f32, tag="ones_t", name="ones_t")
            P.memset(ones_t, 1.0)
            ONESR = ones_t[0:1, :]
            onesq = sb.tile([100, 100], f32, tag="onesq", name="onesq")
            P.memset(onesq, 1.0)
            I100 = i100_t[:, :]
            I20 = i100_t[0:20, 0:20]
            I50 = i100_t[0:50, 0:50]
            IO50 = iota_t[0:100, 0:50]
            IO64 = iota_t[0:50, 0:NRSQ]
            TRIL = tril_t[:, :]
            TRIU = triu_t[:, :]

            # ---- degrees straight off the dst-major grid --------------------
            # GWd's columns are host-permuted so same-hemisphere sources sit in
            # cols 0:50 of every slab -> deg_c is a plain subrange reduce
            dd = sb.tile([100, 2], f32, tag="dd", name="dd")
            gwd3 = GWD.rearrange("p (c j) -> p c j", c=KSLOT)
            V.tensor_reduce(out=dd[:, 0:1], in_=gwd3[:, :, 0:50],
                            axis=mybir.AxisListType.XY, op=Alu.add)
            V.tensor_reduce(out=dd[:, 1:2], in_=gwd3, axis=mybir.AxisListType.XY, op=Alu.add)
            # dis = 1/sqrt(deg+1): +1 self-loop via Sqrt's free bias
            sq2 = sb.tile([100, 2], f32, tag="sq2", name="sq2")
            S.activation(out=sq2, in_=dd, func=Act.Sqrt, bias=1.0)
            # switch Scalar ACT table to the Exp/Tanh set right after the last
            # Sqrt (input dep on sq2 pins the order; the load then overlaps
            # the GCN layers instead of stalling the tail)
            S.activation(out=scr, in_=sq2[0:1, 0:1], func=Act.Tanh)
            rdis = sb.tile([100, 2], f32, tag="rdis", name="rdis")
            V.reciprocal(out=rdis, in_=sq2)
            rdis_c = rdis[:, 0:1]
            rdis_g = rdis[:, 1:2]

            # ---- adjacency slab sums (DVE: gates the layer-1 sandwich) ------
            agtmp = sb.tile([100, 100], f32, tag="agtmp", name="agtmp")
            V.tensor_tensor(out=agtmp, in0=GW[:, 0:100], in1=GW[:, 100:200], op=Alu.add)
            agts = sb.tile([100, 100], f32, tag="agts", name="agts")
            V.tensor_tensor(out=agts, in0=agtmp, in1=GW[:, 200:300], op=Alu.add)
            agt = sb.tile([100, 100], f32, tag="agt", name="agt")
            V.tensor_tensor(out=agt, in0=agts, in1=I100, op=Alu.add)
            act = sb.tile([100, 100], f32, tag="act", name="act")
            V.tensor_tensor(out=act, in0=agt, in1=MBD, op=Alu.mult)
            # ---- dis sandwich for both adjacencies --------------------------
            # row-replicate dis via ones @ diag(dis); diag built on idle GpSimd
            dgc = sb.tile([100, 100], f32, tag="dgc", name="dgc")
            P.affine_select(out=dgc, in_=rdis_c.broadcast_to([100, 100]),
                            compare_op=Alu.is_equal, fill=0.0, base=0,
                            pattern=[[-1, 100]], channel_multiplier=1)
            dgg = sb.tile([100, 100], f32, tag="dgg", name="dgg")
            P.affine_select(out=dgg, in_=rdis_g.broadcast_to([100, 100]),
                            compare_op=Alu.is_equal, fill=0.0, base=0,
                            pattern=[[-1, 100]], channel_multiplier=1)
            drep_c = mm([100, 100], "drep_c")
            T.matmul(drep_c, onesq, dgc)
            V.scalar_tensor_tensor(out=act_s, in0=drep_c, scalar=rdis_c, in1=act,
                                   op0=Alu.mult, op1=Alu.mult)

            # Wc0' = Wc0 - Wc2 (early, off critical path)
            V.tensor_tensor(out=wc0p, in0=WC0, in1=WCC[:, 20:40], op=Alu.subtract)

            # unweighted counts (all ew > 0) on GpSimd; slow there but far off
            # the critical path (first use ~8us later)
            b3 = sb.tile([100, GC], f32, tag="b3", name="b3")
            P.tensor_scalar(out=b3, in0=GW, scalar1=0.0, scalar2=None, op0=Alu.is_gt)
            a1tmp = sb.tile([100, 100], f32, tag="a1tmp", name="a1tmp")
            P.tensor_tensor(out=a1tmp, in0=b3[:, 0:100], in1=b3[:, 100:200], op=Alu.add)
            a1t = sb.tile([100, 100], f32, tag="a1t", name="a1t")
            P.tensor_tensor(out=a1t, in0=a1tmp, in1=b3[:, 200:300], op=Alu.add)

            # ---- layer 1 (out feature-major [64,100]) -----------------------
            xw_ps = mm([100, 64], "xw_ps")
            T.matmul(xw_ps, XTL, W1[:, 0:64], start=True, stop=False)
            T.matmul(xw_ps, XTR, W1[:, 64:128], start=False, stop=True)
            V.tensor_copy(out=AUG[0:100, A_Y1:A_Y1 + 64], in_=xw_ps)
            z1T = mm([64, 100], "z1T")
            T.matmul(z1T, y1aug, acts_aug)
            z1s = sb.tile([64, 100], f32, tag="z1s", name="z1s")
            V.tensor_copy(out=z1s, in_=z1T)
            h1t = sb.tile([64, 100], f32, tag="h1t", name="h1t")
            V.scalar_tensor_tensor(out=h1t, in0=z1s, scalar=0.01, in1=z1s,
                                   op0=Alu.mult, op1=Alu.max)

            # ---- layer 2 ----------------------------------------------------
            xw2 = mm([100, 40], "xw2")
            T.matmul(xw2, h1t, W2)
            y2r = sb.tile([100, 20], f32, tag="y2r", name="y2r")
            V.tensor_scalar_mul(y2r, xw2[:, 20:40], MKR)
            V.scalar_tensor_tensor(out=AUG[0:100, A_Y2:A_Y2 + 20], in0=xw2[:, 0:20],
                                   scalar=MKL, in1=y2r, op0=Alu.mult, op1=Alu.add)
            # global-layer sandwich (first use zgT, ~2us out; V slot after the
            # layer-2 select keeps it clear of the lrelu pairs)
            drep_g = mm([100, 100], "drep_g")
            T.matmul(drep_g, onesq, dgg)
            V.scalar_tensor_tensor(out=agt_s, in0=drep_g, scalar=rdis_g, in1=agt,
                                   op0=Alu.mult, op1=Alu.mult)
            z2T = mm([20, 100], "z2T")
            T.matmul(z2T, y2aug, acts_aug)
            z2s = sb.tile([20, 100], f32, tag="z2s", name="z2s")
            V.tensor_copy(out=z2s, in_=z2T)
            h2at = sb.tile([20, 100], f32, tag="h2at", name="h2at")
            V.scalar_tensor_tensor(out=h2at, in0=z2s, scalar=0.01, in1=z2s,
                                   op0=Alu.mult, op1=Alu.max)

            # ---- global GCN layer ------------------------------------------
            xwg = mm([100, 20], "xwg")
            T.matmul(xwg, h2at, WG)
            V.tensor_copy(out=AUG[0:100, A_YG:A_YG + 20], in_=xwg)
            zgT = mm([20, 100], "zgT")
            T.matmul(zgT, ygaug, agts_aug)
            zgs = sb.tile([20, 100], f32, tag="zgs", name="zgs")
            V.tensor_copy(out=zgs, in_=zgT)
            V.scalar_tensor_tensor(out=h2T, in0=zgs, scalar=0.01, in1=zgs,
                                   op0=Alu.mult, op1=Alu.max)

            # ---- SAGPool score (critical: emitted before h2x/Cheb mms) ------
            h2x = sb.tile([100, 21], f32, tag="h2x", name="h2x")
            score = h2x[:, 20:21]
            hw_ps = mm([100, 2], "hw_ps")
            T.matmul(hw_ps, h2T, WRR2)
            hw = sb.tile([100, 2], f32, tag="hw", name="hw")
            V.tensor_copy(out=hw, in_=hw_ps)
            sc_ps = mm([100, 1], "sc_ps")
            T.matmul(sc_ps, a1t, hw[:, 0:1])
            V.tensor_tensor(out=score, in0=sc_ps, in1=hw[:, 1:2], op=Alu.add)

            # h2 node-major into h2x cols 0:20; Cheb products (need only h2T)
            h2x_p = mm([100, 20], "h2x_p")
            T.transpose(h2x_p, h2T, I20)
            V.tensor_copy(out=h2x[:, 0:20], in_=h2x_p)
            pp_ps = mm([100, 40], "pp_ps")
            T.matmul(pp_ps, h2T, WCC)
            pp = sb.tile([50, 40], f32, tag="pp", name="pp")
            V.tensor_copy(out=pp, in_=pp_ps[0:50, :])
            sraw_ps = mm([100, 20], "sraw_ps")
            T.matmul(sraw_ps, h2Taug, wc0paug, start=True, stop=False)

            # ---- rank / top-k ----------------------------------------------
            dgs = sb.tile([100, 100], f32, tag="dgs", name="dgs")
            P.affine_select(out=dgs, in_=score.broadcast_to([100, 100]),
                            compare_op=Alu.is_equal, fill=0.0, base=0,
                            pattern=[[-1, 100]], channel_multiplier=1)
            srep_ps = mm([100, 100], "srep_ps")
            T.matmul(srep_ps, onesq, dgs)
            t2 = sb.tile([100, 100], f32, tag="t2", name="t2")
            V.scalar_tensor_tensor(out=t2, in0=srep_ps, scalar=score, in1=TRIL,
                                   op0=Alu.is_equal, op1=Alu.mult)
            csum = sb.tile([100, 100], f32, tag="csum", name="csum")
            rank = sb.tile([100, 1], f32, tag="rank", name="rank")
            V.scalar_tensor_tensor(out=csum, in0=srep_ps, scalar=score, in1=t2,
                                   op0=Alu.is_gt, op1=Alu.add, accum_out=rank)
            kept = sb.tile([100, 1], f32, tag="kept", name="kept")
            V.tensor_scalar(out=kept, in0=rank, scalar1=49.5, scalar2=None, op0=Alu.is_lt)
            pit = sb.tile([100, 50], f32, tag="pit", name="pit")
            V.tensor_scalar(out=pit, in0=IO50, scalar1=rank, scalar2=None, op0=Alu.is_equal)

            # ---- pooled rows + gather matrix -------------------------------
            w_ps = mm([100, 1], "w_ps")
            T.matmul(w_ps, a1t, kept)
            w_sb = sb.tile([100, 1], f32, tag="w_sb", name="w_sb")
            V.tensor_copy(out=w_sb, in_=w_ps)
            m1 = mm([100, 50], "m1")
            T.matmul(m1, a1t, pit)
            m1s = sb.tile([100, 50], f32, tag="m1s", name="m1s")
            V.tensor_copy(out=m1s, in_=m1)
            degc_p = mm([50, 1], "degc_p")
            T.matmul(degc_p, pit, w_sb)
            atilt_p = mm([50, 50], "atilt_p")
            T.matmul(atilt_p, m1s, pit)
            p1 = mm([50, 21], "p1")
            T.matmul(p1, pit, h2x)
            th = sb.tile([50, 1], f32, tag="th", name="th")
            S.activation(out=th, in_=p1[:, 20:21], func=Act.Tanh, bias=BREL[0:50, :], scale=1.0)
            p1s = sb.tile([50, 20], f32, tag="p1s", name="p1s")
            V.tensor_copy(out=p1s, in_=p1[:, 0:20])
            srank_p = mm([100, 1], "srank_p")
            T.matmul(srank_p, TRIU, kept)
            gat = sb.tile([100, 50], f32, tag="gat", name="gat")
            V.scalar_tensor_tensor(out=gat, in0=IO50, scalar=srank_p,
                                   in1=kept.broadcast_to([100, 50]),
                                   op0=Alu.is_equal, op1=Alu.mult)

            # pooled-degree rsqrt via integer one-hot lookup (no Scalar Sqrt)
            ohscr = sb.tile([50, NRSQ], f32, tag="ohscr", name="ohscr")
            V.scalar_tensor_tensor(out=ohscr, in0=IO64, scalar=degc_p, in1=RSQ,
                                   op0=Alu.is_equal, op1=Alu.mult)
            disch = sb.tile([50, 1], f32, tag="disch", name="disch")
            V.tensor_reduce(out=disch, in_=ohscr, axis=AxX, op=Alu.add)
            ndisch = sb.tile([50, 1], f32, tag="ndisch", name="ndisch")
            V.tensor_scalar_mul(ndisch, disch, -1.0)
            dgd = sb.tile([50, 50], f32, tag="dgd", name="dgd")
            P.affine_select(out=dgd, in_=disch.broadcast_to([50, 50]),
                            compare_op=Alu.is_equal, fill=0.0, base=0,
                            pattern=[[-1, 50]], channel_multiplier=1)
            drepd = mm([50, 50], "drepd")
            T.matmul(drepd, onesq[0:50, 0:50], dgd)
            gsx1 = sb.tile([50, 50], f32, tag="gsx1", name="gsx1")
            V.tensor_scalar_mul(gsx1, atilt_p, ndisch)
            gsx = sb.tile([50, 100], f32, tag="gsx", name="gsx")
            V.memset(gsx, 0.0)
            V.tensor_tensor(out=gsx[:, 0:50], in0=drepd, in1=gsx1, op=Alu.mult)

            # ---- Cheb accumulation into sraw -------------------------------
            T.matmul(sraw_ps, gsx, pp[:, 0:20], start=False, stop=False)
            q2_ps = mm([100, 20], "q2_ps")
            T.matmul(q2_ps, gsx, pp[:, 20:40])
            q2x2 = sb.tile([50, 20], f32, tag="q2x2", name="q2x2")
            V.tensor_scalar_mul(q2x2, q2_ps[0:50, :], 2.0)
            T.matmul(sraw_ps, gsx, q2x2, start=False, stop=True)

            # ---- double softmax (normalizations folded) --------------------
            ex1 = sb.tile([100, 20], f32, tag="ex1", name="ex1")
            sum1 = sb.tile([100, 1], f32, tag="sum1", name="sum1")
            S.activation(out=ex1, in_=sraw_ps, func=Act.Exp, accum_out=sum1)
            rc1 = sb.tile([100, 1], f32, tag="rc1", name="rc1")
            V.reciprocal(out=rc1, in_=sum1)
            exr = sb.tile([100, 20], f32, tag="exr", name="exr")
            V.tensor_scalar_mul(exr, ex1, rc1)
            ex2 = sb.tile([100, 20], f32, tag="ex2", name="ex2")
            sum2 = sb.tile([100, 1], f32, tag="sum2", name="sum2")
            S.activation(out=ex2, in_=ex1, func=Act.Exp, scale=rc1, accum_out=sum2)
            rc2 = sb.tile([100, 1], f32, tag="rc2", name="rc2")
            V.reciprocal(out=rc2, in_=sum2)
            s2 = sb.tile([100, 20], f32, tag="s2", name="s2")
            V.tensor_scalar_mul(s2, ex2, rc2)

            # ---- diff-pool tail --------------------------------------------
            # M = gat^T @ ass (runs while softmax-2 is still on Scalar)
            m_ps = mm([50, 20], "m_ps")
            T.matmul(m_ps, gat, exr)
            m_sb = sb.tile([50, 20], f32, tag="m_sb", name="m_sb")
            V.tensor_copy(out=m_sb, in_=m_ps)
            mt_ps = mm([20, 50], "mt_ps")
            T.transpose(mt_ps, m_sb, I50)
            mt = sb.tile([20, 50], f32, tag="mt", name="mt")
            V.tensor_copy(out=mt, in_=mt_ps)
            hc_ps = mm([20, 20], "hc_ps")
            T.matmul(hc_ps, s2, h2x[:, 0:20])
            hc = sb.tile([20, 20], f32, tag="hc", name="hc")
            V.tensor_copy(out=hc, in_=hc_ps)
            g_p = mm([50, 20], "g_p")
            T.matmul(g_p, mt, hc)
            outv = sb.tile([50, 20], f32, tag="outv", name="outv")
            V.scalar_tensor_tensor(out=outv, in0=p1s, scalar=th, in1=g_p,
                                   op0=Alu.mult, op1=Alu.add)
            nc.sync.dma_start(out=out_d.ap(), in_=outv, single_packet=True)

    # walrus single-wait workaround
    orig = nc.to_json_bytes
    def patched(*a, **k):
        import json as _json
        return _json.dumps(_split_multiwaits(_json.loads(orig(*a, **k)))).encode()
    nc.to_json_bytes = patched
    return nc


def _pack(inputs) -> np.ndarray:
    f = lambda k: np.asarray(inputs[k], dtype=np.float32)
    blob = np.zeros((128, C_COLS), dtype=np.float32)

    ei = np.asarray(inputs["edge_index"]).astype(np.int64)
    src, dst = ei[0], ei[1]
    ew = f("edge_attr")
    assert (ew > 0).all(), "zero edge weight breaks grid binarization"
    # scatter edges into duplicate slabs (pure placement; no arithmetic)
    slot = {}
    gwd = np.zeros((100, KSLOT, 100), np.float32)
    gw = np.zeros((100, KSLOT, 100), np.float32)
    for e in range(E):
        s, d = int(src[e]), int(dst[e])
        k = slot.get((s, d), 0)
        slot[(s, d)] = k + 1
        assert k < KSLOT, "duplicate-edge multiplicity exceeds KSLOT"
        # dst-major grid: per-row column rotation puts same-hemisphere
        # sources in cols 0:50 (pure placement)
        sc = s if d < 50 else (s + 50) % 100
        gwd[d, k, sc] = ew[e]
        gw[s, k, d] = ew[e]
    blob[0:100, O_GWD:O_GWD + GC] = gwd.reshape(100, GC)
    blob[0:100, O_GW:O_GW + GC] = gw.reshape(100, GC)

    half = np.arange(100) < 50
    blob[0:100, O_MBD:O_MBD + 100] = (half[:, None] == half[None, :]).astype(np.float32)

    x = f("x")
    xt = x.T.copy()
    xtl = xt.copy(); xtl[:, 50:] = 0.0
    xtr = xt.copy(); xtr[:, :50] = 0.0
    blob[0:100, O_XTL:O_XTL + 100] = xtl
    blob[0:100, O_XTR:O_XTR + 100] = xtr
    blob[0:100, O_W1:O_W1 + 64] = f("Wl1")
    blob[0:100, O_W1 + 64:O_W1 + 128] = f("Wr1")

    blob[0:50, O_MKL] = 1.0
    blob[50:100, O_MKR] = 1.0
    blob[:, O_BREL] = f("brel")[0]
    blob[0:64, O_W2:O_W2 + 20] = f("Wl2")
    blob[0:64, O_W2 + 20:O_W2 + 40] = f("Wr2")
    # 1/sqrt(k) lookup rows (constants; row-replicated for the free-dim dot)
    ks = np.arange(NRSQ, dtype=np.float32)
    rsq = np.zeros(NRSQ, np.float32)
    rsq[1:] = 1.0 / np.sqrt(ks[1:])
    blob[0:50, O_RSQ:O_RSQ + NRSQ] = rsq[None, :]
    blob[0:20, O_WG:O_WG + 20] = f("Wg1")
    blob[0:20, O_WC0:O_WC0 + 20] = f("Wc0")
    blob[0:20, O_WCC:O_WCC + 20] = f("Wc1")
    blob[0:20, O_WCC + 20:O_WCC + 40] = f("Wc2")
    blob[0:20, O_WRR] = f("Wrel")[:, 0]
    blob[0:20, O_WRR + 1] = f("Wroot")[:, 0]
    return blob


def _pack_aug(inputs) -> np.ndarray:
    f = lambda k: np.asarray(inputs[k], dtype=np.float32)
    aug = np.zeros((2, CAUG_COLS), np.float32)
    half = (np.arange(100) < 50).astype(np.float32)
    aug[0, A_Y1:A_Y1 + 64] = f("bl1")
    aug[1, A_Y1:A_Y1 + 64] = f("br1")
    aug[0, A_Y2:A_Y2 + 20] = f("bl2")
    aug[1, A_Y2:A_Y2 + 20] = f("br2")
    aug[0, A_YG:A_YG + 20] = f("bg1")
    aug[0, A_ACT:A_ACT + 100] = half
    aug[1, A_ACT:A_ACT + 100] = 1.0 - half
    aug[0, A_AGT:A_AGT + 100] = 1.0
    aug[0, A_WC0:A_WC0 + 20] = f("bc")
    aug[0, A_H2T:A_H2T + 100] = 1.0
    return aug


_NC = None

def _get_nc():
    global _NC
    if _NC is None:
        _NC = _build()
    return _NC


def run(inputs, trace=False):
    from concourse.bass_utils import run_bass_kernel_spmd
    nc = _get_nc()
    blob = _pack(inputs)
    parts = {
        "inbufD": np.ascontiguousarray(blob[:, 0:C_DMA_D]),
        "inbufA": np.ascontiguousarray(blob[:, C_DMA_D:C_DMA_A]),
        "inbufB": np.ascontiguousarray(blob[:, C_DMA_A:C_DMA_B]),
        "inbufC": np.ascontiguousarray(blob[:, C_DMA_B:C_COLS]),
        "inbufE": _pack_aug(inputs),
    }
    in_maps = [dict(parts) for _ in range(8)]
    res = run_bass_kernel_spmd(nc, in_maps, list(range(8)), trace=trace)
    out = np.asarray(res.results[0]["out"], dtype=np.float32).reshape(1, K1 * 20)
    return out, res


def kernel(**inputs) -> np.ndarray:
    out, _ = run(inputs)
    return out


# revision 97
# speedup vs baseline: 1.2100x; 1.0914x over previous
"""Trainium2 Bass kernel for nn_Brain_connectomic_graph (GNN message passing).

Single tiny graph (N=100 nodes, E=2000 edges); whole network as dense linear
algebra on ONE NeuronCore, replicated across 8 cores (data-parallel lanes,
batch=1 per the sharding hint); core 0's output is returned.

v3 design (latency-focused):
  - Adjacency densification done on the HOST as pure data placement: edges
    scattered into K=3 duplicate-slab grids (a duplicate (src,dst) pair goes
    to the next slab; no host arithmetic). Device sums slabs with 2 adds.
  - No unweighted grid: A1 (counts) comes from binarizing the weighted grid
    on GpSimd (all edge weights are nonzero).
  - No grid diagonal: the GCN +1 self-loop degree enters via the Sqrt
    activation's free bias; the +I adjacency term via one add with the
    on-device identity.
  - Degrees come from the dst-major grid via free-axis reduces (V only).
  - GCN layers alternate node-major/feature-major layouts -> NO transposes
    between layers; hemisphere selection via host-masked X^T stationaries
    (layer 1) and a 2-op DVE select (layer 2).
  - Layer biases enter as EXTRA CONTRACTION ROWS: stationaries/movings are
    augmented to k=101/102 with [bias rows | hemisphere masks], so bias
    needs no separate matmul or vector op anywhere.
  - dis sandwich built once per adjacency (shared by both layers).
  - ChebConv reassociated: s_raw = h2@(Wc0-Wc2) + G@(h2@Wc1) + 2G@(G@(h2@Wc2))
    with G the sandwiched pooled adjacency -- no Tx transposes.
  - Pooled-degree rsqrt via integer one-hot lookup against a host 1/sqrt(k)
    table (2 DVE ops, no Scalar Sqrt mid-kernel).
  - Scalar ACT tables: Sqrt set prewarmed during DMA; Exp/Tanh set loaded
    right after the single early Sqrt -- no stalls later.
"""

import numpy as np

N = 100
E = 2000
K1 = 50
KSLOT = 3          # duplicate-edge slabs (max multiplicity in data is 3)
GC = KSLOT * 100   # grid columns

# ---- inbuf column layout (f32 blob [128, C]) --------------------------------
_off = 0
def _nxt(w):
    global _off
    o = _off
    _off += w
    return o

# DMA group D (first: gates the degree/dis chain)
O_GWD  = _nxt(GC)     # [100,3,100] GWd[d, k, s] = ew(s->d), no diag
O_MBD  = _nxt(100)    # [100,100] same-hemisphere block mask
C_DMA_D = _off
# DMA group A
O_GW   = _nxt(GC)     # [100,3,100] GW[s, k, d] = ew(s->d), no diag
C_DMA_A = _off
# DMA group B: first-matmul operands
O_XTL  = _nxt(100)    # [100,100] x^T with cols (nodes) >=50 zeroed
O_XTR  = _nxt(100)    # [100,100] x^T with cols (nodes) <50 zeroed
O_W1   = _nxt(128)    # [100,128] [Wl1 | Wr1]
C_DMA_B = _off
# DMA group C: small weights/tables
O_MKL  = _nxt(1)      # [100,1] 1.0 for p<50
O_MKR  = _nxt(1)      # [100,1] 1.0 for 50<=p<100
O_BREL = _nxt(1)      # [128,1] brel broadcast
O_W2   = _nxt(40)     # [64,40] [Wl2|Wr2]
O_RSQ  = _nxt(64)     # [50,64] 1/sqrt(k) lookup rows
O_WG   = _nxt(20)     # [20,20] Wg1
O_WC0  = _nxt(20)     # [20,20] Wc0
O_WCC  = _nxt(40)     # [20,40] [Wc1 | Wc2]
O_WRR  = _nxt(2)      # [20,2]  [Wrel | Wroot]
C_COLS = _off
NRSQ = 64

# AUG mega-tile column layout: all bias-augmented stationaries/movings live in
# one [128, CAUG_COLS] tile so a single DMA fills every aug row (rows 100:102)
A_Y1, A_Y2, A_YG, A_ACT, A_AGT, A_WC0, A_H2T = 0, 64, 84, 104, 204, 304, 324
CAUG_COLS = 424


def _split_multiwaits(bir: dict) -> dict:
    """This container's walrus accepts only ONE sync-wait per instruction.
    Insert single-wait NoOps (same engine, just before) for the extras."""
    for f in bir.get("functions", []):
        for bb in f.get("blocks", []):
            out = []
            for ins in bb.get("instructions", []):
                si = ins.get("sync_info")
                waits = (si or {}).get("on_wait") or []
                if len(waits) > 1:
                    for i, w in enumerate(waits[:-1]):
                        out.append({
                            "debug": ins.get("debug", 0),
                            "engine": ins["engine"],
                            "ins": [], "outs": [],
                            "name": f"{ins['name']}-w{i}",
                            "opcode": "NoOp",
                            "sync_info": {"on_wait": [w], "on_update": []},
                        })
                    si["on_wait"] = [waits[-1]]
                out.append(ins)
            bb["instructions"] = out
    return bir


def _build():
    import concourse.bass as bass
    import concourse.mybir as mybir
    import concourse.tile as tile

    f32 = mybir.dt.float32
    Alu = mybir.AluOpType
    Act = mybir.ActivationFunctionType
    AxX = mybir.AxisListType.X

    nc = bass.Bass("TRN2")
    in_d = nc.dram_tensor("inbufD", [128, C_DMA_D], f32, kind="ExternalInput")
    in_a = nc.dram_tensor("inbufA", [128, C_DMA_A - C_DMA_D], f32, kind="ExternalInput")
    in_b = nc.dram_tensor("inbufB", [128, C_DMA_B - C_DMA_A], f32, kind="ExternalInput")
    in_c = nc.dram_tensor("inbufC", [128, C_COLS - C_DMA_B], f32, kind="ExternalInput")
    in_e = nc.dram_tensor("inbufE", [2, CAUG_COLS], f32, kind="ExternalInput")
    out_d = nc.dram_tensor("out", [K1, 20], f32, kind="ExternalOutput")

    with tile.TileContext(nc) as tc:
        with (
            tc.tile_pool(name="sb", bufs=1) as sb,
            tc.tile_pool(name="ps", bufs=1, space="PSUM") as ps,
        ):
            ibD = sb.tile([128, C_DMA_D], f32, tag="ibD", name="ibD")
            nc.sync.dma_start(out=ibD[:, :], in_=in_d.ap())
            ibA = sb.tile([128, C_DMA_A - C_DMA_D], f32, tag="ibA", name="ibA")
            nc.sync.dma_start(out=ibA[:, :], in_=in_a.ap())
            ibB = sb.tile([128, C_DMA_B - C_DMA_A], f32, tag="ibB", name="ibB")
            nc.sync.dma_start(out=ibB[:, :], in_=in_b.ap())
            ibC = sb.tile([128, C_COLS - C_DMA_B], f32, tag="ibC", name="ibC")
            nc.sync.dma_start(out=ibC[:, :], in_=in_c.ap())

            GWD  = ibD[0:100, O_GWD:O_GWD + GC]
            MBD  = ibD[0:100, O_MBD:O_MBD + 100]
            GW   = ibA[0:100, 0:GC]
            XTL  = ibB[0:100, O_XTL - C_DMA_A:O_XTL - C_DMA_A + 100]
            XTR  = ibB[0:100, O_XTR - C_DMA_A:O_XTR - C_DMA_A + 100]
            W1   = ibB[0:100, O_W1 - C_DMA_A:O_W1 - C_DMA_A + 128]
            def icl(off, w, p0=0, p1=128):
                return ibC[p0:p1, off - C_DMA_B:off - C_DMA_B + w]
            MKL  = icl(O_MKL, 1, 0, 100)
            MKR  = icl(O_MKR, 1, 0, 100)
            BREL = icl(O_BREL, 1)
            W2   = icl(O_W2, 40, 0, 64)
            RSQ  = icl(O_RSQ, NRSQ, 0, 50)
            WG   = icl(O_WG, 20, 0, 20)
            WC0  = icl(O_WC0, 20, 0, 20)
            WCC  = icl(O_WCC, 40, 0, 20)
            WRR2 = icl(O_WRR, 2, 0, 20)

            V = nc.vector
            S = nc.scalar
            P = nc.gpsimd
            T = nc.tensor
            mm = lambda shape, name: ps.tile(shape, f32, tag="mm", name=name, bufs=7)

            # AUG mega-tile: all bias-augmented operands; zeroed once, aug rows
            # (100:102) filled by ONE small DMA
            AUG = sb.tile([128, CAUG_COLS], f32, tag="AUG", name="AUG")
            V.memset(AUG, 0.0)
            nc.sync.dma_start(out=AUG[100:102, :], in_=in_e.ap())
            y1aug = AUG[0:102, A_Y1:A_Y1 + 64]
            y2aug = AUG[0:102, A_Y2:A_Y2 + 20]
            ygaug = AUG[0:101, A_YG:A_YG + 20]
            acts_aug = AUG[0:102, A_ACT:A_ACT + 100]
            agts_aug = AUG[0:101, A_AGT:A_AGT + 100]
            wc0paug = AUG[0:102, A_WC0:A_WC0 + 20]
            h2Taug = AUG[0:102, A_H2T:A_H2T + 100]
            act_s = AUG[0:100, A_ACT:A_ACT + 100]
            agt_s = AUG[0:100, A_AGT:A_AGT + 100]
            h2T = AUG[0:20, A_H2T:A_H2T + 100]
            wc0p = AUG[0:20, A_WC0:A_WC0 + 20]

            # ---- prologue: ACT sqrt-set prewarm + PE warmup (HAM ramp) ------
            scr = sb.tile([1, 1], f32, tag="scr", name="scr")
            V.memset(scr, 0.0)
            S.activation(out=scr, in_=scr, func=Act.Sqrt)
            wmt = sb.tile([128, 100], f32, tag="wmt", name="wmt")
            V.memset(wmt, 1.0)
            warm = ps.tile([100, 200], f32, tag="warm", name="warm", bufs=1)
            wm_b = wmt.unsqueeze(1).broadcast_to([128, 2, 100])
            for _ in range(4):
                T.matmul(warm, wmt, wm_b)

            # ---- on-device constants (GpSimd, runs during the DMAs) ---------
            iota_i = sb.tile([128, 100], mybir.dt.int32, tag="iota_i", name="iota_i")
            P.iota(iota_i, pattern=[[1, 100]], base=0, channel_multiplier=0)
            iota_t = sb.tile([128, 100], f32, tag="iota_t", name="iota_t")
            P.tensor_copy(out=iota_t, in_=iota_i)
            i100_t = sb.tile([100, 100], f32, tag="i100_t", name="i100_t")
            P.memset(i100_t, 0.0)
            P.affine_select(out=i100_t, in_=i100_t, compare_op=Alu.not_equal,
                            fill=1.0, base=0, pattern=[[-1, 100]], channel_multiplier=1)
            tril_t = sb.tile([100, 100], f32, tag="tril_t", name="tril_t")
            P.memset(tril_t, 1.0)
            P.affine_select(out=tril_t, in_=tril_t, compare_op=Alu.is_gt,
                            fill=0.0, base=0, pattern=[[-1, 100]], channel_multiplier=1)
            triu_t = sb.tile([100, 100], f32, tag="triu_t", name="triu_t")
            P.memset(triu_t, 1.0)
            P.affine_select(out=triu_t, in_=triu_t, compare_op=Alu.is_gt,
                            fill=0.0, base=0, pattern=[[1, 100]], channel_multiplier=-1)
            ones_t = sb.tile([1, 100], f32, tag="ones_t", name="ones_t")
            P.memset(ones_t, 1.0)
            ONESR = ones_t[0:1, :]
            onesq = sb.tile([100, 100], f32, tag="onesq", name="onesq")
            P.memset(onesq, 1.0)
            I100 = i100_t[:, :]
            I20 = i100_t[0:20, 0:20]
            I50 = i100_t[0:50, 0:50]
            IO50 = iota_t[0:100, 0:50]
            IO64 = iota_t[0:50, 0:NRSQ]
            TRIL = tril_t[:, :]
            TRIU = triu_t[:, :]

            # ---- degrees straight off the dst-major grid --------------------
            # GWd's columns are host-permuted so same-hemisphere sources sit in
            # cols 0:50 of every slab -> deg_c is a plain subrange reduce
            dd = sb.tile([100, 2], f32, tag="dd", name="dd")
            gwd3 = GWD.rearrange("p (c j) -> p c j", c=KSLOT)
            V.tensor_reduce(out=dd[:, 0:1], in_=gwd3[:, :, 0:50],
                            axis=mybir.AxisListType.XY, op=Alu.add)
            V.tensor_reduce(out=dd[:, 1:2], in_=gwd3, axis=mybir.AxisListType.XY, op=Alu.add)
            # dis = 1/sqrt(deg+1): +1 self-loop via Sqrt's free bias
            sq2 = sb.tile([100, 2], f32, tag="sq2", name="sq2")
            S.activation(out=sq2, in_=dd, func=Act.Sqrt, bias=1.0)
            # switch Scalar ACT table to the Exp/Tanh set right after the last
            # Sqrt (input dep on sq2 pins the order; the load then overlaps
            # the GCN layers instead of stalling the tail)
            S.activation(out=scr, in_=sq2[0:1, 0:1], func=Act.Tanh)
            rdis = sb.tile([100, 2], f32, tag="rdis", name="rdis")
            V.reciprocal(out=rdis, in_=sq2)
            rdis_c = rdis[:, 0:1]
            rdis_g = rdis[:, 1:2]

            # diag(dis_c) on GpSimd BEFORE the slab adds: it gates the layer-1
            # sandwich matmul while the adds have more slack
            dgc = sb.tile([100, 100], f32, tag="dgc", name="dgc")
            P.affine_select(out=dgc, in_=rdis_c.broadcast_to([100, 100]),
                            compare_op=Alu.is_equal, fill=0.0, base=0,
                            pattern=[[-1, 100]], channel_multiplier=1)

            # ---- adjacency slab sums (GpSimd; DVE stays on the deg chain) ---
            agtmp = sb.tile([100, 100], f32, tag="agtmp", name="agtmp")
            P.tensor_tensor(out=agtmp, in0=GW[:, 0:100], in1=GW[:, 100:200], op=Alu.add)
            agts = sb.tile([100, 100], f32, tag="agts", name="agts")
            P.tensor_tensor(out=agts, in0=agtmp, in1=GW[:, 200:300], op=Alu.add)
            agt = sb.tile([100, 100], f32, tag="agt", name="agt")
            P.tensor_tensor(out=agt, in0=agts, in1=I100, op=Alu.add)
            act = sb.tile([100, 100], f32, tag="act", name="act")
            P.tensor_tensor(out=act, in0=agt, in1=MBD, op=Alu.mult)
            # ---- dis sandwich for both adjacencies --------------------------
            # row-replicate dis via ones @ diag(dis)
            dgg = sb.tile([100, 100], f32, tag="dgg", name="dgg")
            P.affine_select(out=dgg, in_=rdis_g.broadcast_to([100, 100]),
                            compare_op=Alu.is_equal, fill=0.0, base=0,
                            pattern=[[-1, 100]], channel_multiplier=1)
            drep_c = mm([100, 100], "drep_c")
            T.matmul(drep_c, onesq, dgc)
            V.scalar_tensor_tensor(out=act_s, in0=drep_c, scalar=rdis_c, in1=act,
                                   op0=Alu.mult, op1=Alu.mult)

            # Wc0' = Wc0 - Wc2 (early, off critical path)
            V.tensor_tensor(out=wc0p, in0=WC0, in1=WCC[:, 20:40], op=Alu.subtract)

            # unweighted counts (all ew > 0)
            b3 = sb.tile([100, GC], f32, tag="b3", name="b3")
            V.tensor_scalar(out=b3, in0=GW, scalar1=0.0, scalar2=None, op0=Alu.is_gt)
            a1tmp = sb.tile([100, 100], f32, tag="a1tmp", name="a1tmp")
            V.tensor_tensor(out=a1tmp, in0=b3[:, 0:100], in1=b3[:, 100:200], op=Alu.add)
            a1t = sb.tile([100, 100], f32, tag="a1t", name="a1t")
            V.tensor_tensor(out=a1t, in0=a1tmp, in1=b3[:, 200:300], op=Alu.add)

            # ---- layer 1 (out feature-major [64,100]) -----------------------
            xw_ps = mm([100, 64], "xw_ps")
            T.matmul(xw_ps, XTL, W1[:, 0:64], start=True, stop=False)
            T.matmul(xw_ps, XTR, W1[:, 64:128], start=False, stop=True)
            V.tensor_copy(out=AUG[0:100, A_Y1:A_Y1 + 64], in_=xw_ps)
            z1T = mm([64, 100], "z1T")
            T.matmul(z1T, y1aug, acts_aug)
            # global-layer sandwich off the critical path (first use ~5us out)
            drep_g = mm([100, 100], "drep_g")
            T.matmul(drep_g, onesq, dgg)
            V.scalar_tensor_tensor(out=agt_s, in0=drep_g, scalar=rdis_g, in1=agt,
                                   op0=Alu.mult, op1=Alu.mult)
            z1s = sb.tile([64, 100], f32, tag="z1s", name="z1s")
            V.tensor_copy(out=z1s, in_=z1T)
            h1t = sb.tile([64, 100], f32, tag="h1t", name="h1t")
            V.scalar_tensor_tensor(out=h1t, in0=z1s, scalar=0.01, in1=z1s,
                                   op0=Alu.mult, op1=Alu.max)

            # ---- layer 2 ----------------------------------------------------
            xw2 = mm([100, 40], "xw2")
            T.matmul(xw2, h1t, W2)
            y2r = sb.tile([100, 20], f32, tag="y2r", name="y2r")
            V.tensor_scalar_mul(y2r, xw2[:, 20:40], MKR)
            V.scalar_tensor_tensor(out=AUG[0:100, A_Y2:A_Y2 + 20], in0=xw2[:, 0:20],
                                   scalar=MKL, in1=y2r, op0=Alu.mult, op1=Alu.add)
            z2T = mm([20, 100], "z2T")
            T.matmul(z2T, y2aug, acts_aug)
            z2s = sb.tile([20, 100], f32, tag="z2s", name="z2s")
            V.tensor_copy(out=z2s, in_=z2T)
            h2at = sb.tile([20, 100], f32, tag="h2at", name="h2at")
            V.scalar_tensor_tensor(out=h2at, in0=z2s, scalar=0.01, in1=z2s,
                                   op0=Alu.mult, op1=Alu.max)

            # ---- global GCN layer ------------------------------------------
            xwg = mm([100, 20], "xwg")
            T.matmul(xwg, h2at, WG)
            V.tensor_copy(out=AUG[0:100, A_YG:A_YG + 20], in_=xwg)
            zgT = mm([20, 100], "zgT")
            T.matmul(zgT, ygaug, agts_aug)
            zgs = sb.tile([20, 100], f32, tag="zgs", name="zgs")
            V.tensor_copy(out=zgs, in_=zgT)
            V.scalar_tensor_tensor(out=h2T, in0=zgs, scalar=0.01, in1=zgs,
                                   op0=Alu.mult, op1=Alu.max)

            # ---- SAGPool score (critical: emitted before h2x/Cheb mms) ------
            h2x = sb.tile([100, 21], f32, tag="h2x", name="h2x")
            score = h2x[:, 20:21]
            hw_ps = mm([100, 2], "hw_ps")
            T.matmul(hw_ps, h2T, WRR2)
            hw = sb.tile([100, 2], f32, tag="hw", name="hw")
            V.tensor_copy(out=hw, in_=hw_ps)
            sc_ps = mm([100, 1], "sc_ps")
            T.matmul(sc_ps, a1t, hw[:, 0:1])
            V.tensor_tensor(out=score, in0=sc_ps, in1=hw[:, 1:2], op=Alu.add)

            # h2 node-major into h2x cols 0:20; Cheb products (need only h2T)
            h2x_p = mm([100, 20], "h2x_p")
            T.transpose(h2x_p, h2T, I20)
            V.tensor_copy(out=h2x[:, 0:20], in_=h2x_p)
            pp_ps = mm([100, 40], "pp_ps")
            T.matmul(pp_ps, h2T, WCC)
            pp = sb.tile([50, 40], f32, tag="pp", name="pp")
            V.tensor_copy(out=pp, in_=pp_ps[0:50, :])
            sraw_ps = mm([100, 20], "sraw_ps")
            T.matmul(sraw_ps, h2Taug, wc0paug, start=True, stop=False)

            # ---- rank / top-k ----------------------------------------------
            dgs = sb.tile([100, 100], f32, tag="dgs", name="dgs")
            P.affine_select(out=dgs, in_=score.broadcast_to([100, 100]),
                            compare_op=Alu.is_equal, fill=0.0, base=0,
                            pattern=[[-1, 100]], channel_multiplier=1)
            srep_ps = mm([100, 100], "srep_ps")
            T.matmul(srep_ps, onesq, dgs)
            t2 = sb.tile([100, 100], f32, tag="t2", name="t2")
            V.scalar_tensor_tensor(out=t2, in0=srep_ps, scalar=score, in1=TRIL,
                                   op0=Alu.is_equal, op1=Alu.mult)
            csum = sb.tile([100, 100], f32, tag="csum", name="csum")
            rank = sb.tile([100, 1], f32, tag="rank", name="rank")
            V.scalar_tensor_tensor(out=csum, in0=srep_ps, scalar=score, in1=t2,
                                   op0=Alu.is_gt, op1=Alu.add, accum_out=rank)
            kept = sb.tile([100, 1], f32, tag="kept", name="kept")
            V.tensor_scalar(out=kept, in0=rank, scalar1=49.5, scalar2=None, op0=Alu.is_lt)
            pit = sb.tile([100, 50], f32, tag="pit", name="pit")
            V.tensor_scalar(out=pit, in0=IO50, scalar1=rank, scalar2=None, op0=Alu.is_equal)

            # ---- pooled rows + gather matrix -------------------------------
            w_ps = mm([100, 1], "w_ps")
            T.matmul(w_ps, a1t, kept)
            w_sb = sb.tile([100, 1], f32, tag="w_sb", name="w_sb")
            V.tensor_copy(out=w_sb, in_=w_ps)
            m1 = mm([100, 50], "m1")
            T.matmul(m1, a1t, pit)
            m1s = sb.tile([100, 50], f32, tag="m1s", name="m1s")
            V.tensor_copy(out=m1s, in_=m1)
            degc_p = mm([50, 1], "degc_p")
            T.matmul(degc_p, pit, w_sb)
            atilt_p = mm([50, 50], "atilt_p")
            T.matmul(atilt_p, m1s, pit)
            p1 = mm([50, 21], "p1")
            T.matmul(p1, pit, h2x)
            th = sb.tile([50, 1], f32, tag="th", name="th")
            S.activation(out=th, in_=p1[:, 20:21], func=Act.Tanh, bias=BREL[0:50, :], scale=1.0)
            p1s = sb.tile([50, 20], f32, tag="p1s", name="p1s")
            V.tensor_copy(out=p1s, in_=p1[:, 0:20])
            srank_p = mm([100, 1], "srank_p")
            T.matmul(srank_p, TRIU, kept)
            gat = sb.tile([100, 50], f32, tag="gat", name="gat")
            V.scalar_tensor_tensor(out=gat, in0=IO50, scalar=srank_p,
                                   in1=kept.broadcast_to([100, 50]),
                                   op0=Alu.is_equal, op1=Alu.mult)

            # pooled-degree rsqrt via integer one-hot lookup (no Scalar Sqrt)
            ohscr = sb.tile([50, NRSQ], f32, tag="ohscr", name="ohscr")
            V.scalar_tensor_tensor(out=ohscr, in0=IO64, scalar=degc_p, in1=RSQ,
                                   op0=Alu.is_equal, op1=Alu.mult)
            disch = sb.tile([50, 1], f32, tag="disch", name="disch")
            V.tensor_reduce(out=disch, in_=ohscr, axis=AxX, op=Alu.add)
            ndisch = sb.tile([50, 1], f32, tag="ndisch", name="ndisch")
            V.tensor_scalar_mul(ndisch, disch, -1.0)
            dgd = sb.tile([50, 50], f32, tag="dgd", name="dgd")
            P.affine_select(out=dgd, in_=disch.broadcast_to([50, 50]),
                            compare_op=Alu.is_equal, fill=0.0, base=0,
                            pattern=[[-1, 50]], channel_multiplier=1)
            drepd = mm([50, 50], "drepd")
            T.matmul(drepd, onesq[0:50, 0:50], dgd)
            gsx1 = sb.tile([50, 50], f32, tag="gsx1", name="gsx1")
            V.tensor_scalar_mul(gsx1, atilt_p, ndisch)
            gsx = sb.tile([50, 100], f32, tag="gsx", name="gsx")
            V.memset(gsx, 0.0)
            V.tensor_tensor(out=gsx[:, 0:50], in0=drepd, in1=gsx1, op=Alu.mult)

            # ---- Cheb accumulation into sraw -------------------------------
            T.matmul(sraw_ps, gsx, pp[:, 0:20], start=False, stop=False)
            q2_ps = mm([100, 20], "q2_ps")
            T.matmul(q2_ps, gsx, pp[:, 20:40])
            q2x2 = sb.tile([50, 20], f32, tag="q2x2", name="q2x2")
            V.tensor_scalar_mul(q2x2, q2_ps[0:50, :], 2.0)
            T.matmul(sraw_ps, gsx, q2x2, start=False, stop=True)

            # ---- double softmax (normalizations folded) --------------------
            ex1 = sb.tile([100, 20], f32, tag="ex1", name="ex1")
            sum1 = sb.tile([100, 1], f32, tag="sum1", name="sum1")
            S.activation(out=ex1, in_=sraw_ps, func=Act.Exp, accum_out=sum1)
            rc1 = sb.tile([100, 1], f32, tag="rc1", name="rc1")
            V.reciprocal(out=rc1, in_=sum1)
            exr = sb.tile([100, 20], f32, tag="exr", name="exr")
            V.tensor_scalar_mul(exr, ex1, rc1)
            ex2 = sb.tile([100, 20], f32, tag="ex2", name="ex2")
            sum2 = sb.tile([100, 1], f32, tag="sum2", name="sum2")
            S.activation(out=ex2, in_=ex1, func=Act.Exp, scale=rc1, accum_out=sum2)
            rc2 = sb.tile([100, 1], f32, tag="rc2", name="rc2")
            V.reciprocal(out=rc2, in_=sum2)
            s2 = sb.tile([100, 20], f32, tag="s2", name="s2")
            V.tensor_scalar_mul(s2, ex2, rc2)

            # ---- diff-pool tail --------------------------------------------
            # M = gat^T @ ass (runs while softmax-2 is still on Scalar)
            m_ps = mm([50, 20], "m_ps")
            T.matmul(m_ps, gat, exr)
            m_sb = sb.tile([50, 20], f32, tag="m_sb", name="m_sb")
            V.tensor_copy(out=m_sb, in_=m_ps)
            mt_ps = mm([20, 50], "mt_ps")
            T.transpose(mt_ps, m_sb, I50)
            mt = sb.tile([20, 50], f32, tag="mt", name="mt")
            V.tensor_copy(out=mt, in_=mt_ps)
            hc_ps = mm([20, 20], "hc_ps")
            T.matmul(hc_ps, s2, h2x[:, 0:20])
            hc = sb.tile([20, 20], f32, tag="hc", name="hc")
            V.tensor_copy(out=hc, in_=hc_ps)
            g_p = mm([50, 20], "g_p")
            T.matmul(g_p, mt, hc)
            outv = sb.tile([50, 20], f32, tag="outv", name="outv")
            V.scalar_tensor_tensor(out=outv, in0=p1s, scalar=th, in1=g_p,
                                   op0=Alu.mult, op1=Alu.add)
            nc.sync.dma_start(out=out_d.ap(), in_=outv, single_packet=True)

    # walrus single-wait workaround
    orig = nc.to_json_bytes
    def patched(*a, **k):
        import json as _json
        return _json.dumps(_split_multiwaits(_json.loads(orig(*a, **k)))).encode()
    nc.to_json_bytes = patched
    return nc


def _pack(inputs) -> np.ndarray:
    f = lambda k: np.asarray(inputs[k], dtype=np.float32)
    blob = np.zeros((128, C_COLS), dtype=np.float32)

    ei = np.asarray(inputs["edge_index"]).astype(np.int64)
    src, dst = ei[0], ei[1]
    ew = f("edge_attr")
    assert (ew > 0).all(), "zero edge weight breaks grid binarization"
    # scatter edges into duplicate slabs (pure placement; no arithmetic)
    slot = {}
    gwd = np.zeros((100, KSLOT, 100), np.float32)
    gw = np.zeros((100, KSLOT, 100), np.float32)
    for e in range(E):
        s, d = int(src[e]), int(dst[e])
        k = slot.get((s, d), 0)
        slot[(s, d)] = k + 1
        assert k < KSLOT, "duplicate-edge multiplicity exceeds KSLOT"
        # dst-major grid: per-row column rotation puts same-hemisphere
        # sources in cols 0:50 (pure placement)
        sc = s if d < 50 else (s + 50) % 100
        gwd[d, k, sc] = ew[e]
        gw[s, k, d] = ew[e]
    blob[0:100, O_GWD:O_GWD + GC] = gwd.reshape(100, GC)
    blob[0:100, O_GW:O_GW + GC] = gw.reshape(100, GC)

    half = np.arange(100) < 50
    blob[0:100, O_MBD:O_MBD + 100] = (half[:, None] == half[None, :]).astype(np.float32)

    x = f("x")
    xt = x.T.copy()
    xtl = xt.copy(); xtl[:, 50:] = 0.0
    xtr = xt.copy(); xtr[:, :50] = 0.0
    blob[0:100, O_XTL:O_XTL + 100] = xtl
    blob[0:100, O_XTR:O_XTR + 100] = xtr
    blob[0:100, O_W1:O_W1 + 64] = f("Wl1")
    blob[0:100, O_W1 + 64:O_W1 + 128] = f("Wr1")

    blob[0:50, O_MKL] = 1.0
    blob[50:100, O_MKR] = 1.0
    blob[:, O_BREL] = f("brel")[0]
    blob[0:64, O_W2:O_W2 + 20] = f("Wl2")
    blob[0:64, O_W2 + 20:O_W2 + 40] = f("Wr2")
    # 1/sqrt(k) lookup rows (constants; row-replicated for the free-dim dot)
    ks = np.arange(NRSQ, dtype=np.float32)
    rsq = np.zeros(NRSQ, np.float32)
    rsq[1:] = 1.0 / np.sqrt(ks[1:])
    blob[0:50, O_RSQ:O_RSQ + NRSQ] = rsq[None, :]
    blob[0:20, O_WG:O_WG + 20] = f("Wg1")
    blob[0:20, O_WC0:O_WC0 + 20] = f("Wc0")
    blob[0:20, O_WCC:O_WCC + 20] = f("Wc1")
    blob[0:20, O_WCC + 20:O_WCC + 40] = f("Wc2")
    blob[0:20, O_WRR] = f("Wrel")[:, 0]
    blob[0:20, O_WRR + 1] = f("Wroot")[:, 0]
    return blob


def _pack_aug(inputs) -> np.ndarray:
    f = lambda k: np.asarray(inputs[k], dtype=np.float32)
    aug = np.zeros((2, CAUG_COLS), np.float32)
    half = (np.arange(100) < 50).astype(np.float32)
    aug[0, A_Y1:A_Y1 + 64] = f("bl1")
    aug[1, A_Y1:A_Y1 + 64] = f("br1")
    aug[0, A_Y2:A_Y2 + 20] = f("bl2")
    aug[1, A_Y2:A_Y2 + 20] = f("br2")
    aug[0, A_YG:A_YG + 20] = f("bg1")
    aug[0, A_ACT:A_ACT + 100] = half
    aug[1, A_ACT:A_ACT + 100] = 1.0 - half
    aug[0, A_AGT:A_AGT + 100] = 1.0
    aug[0, A_WC0:A_WC0 + 20] = f("bc")
    aug[0, A_H2T:A_H2T + 100] = 1.0
    return aug


_NC = None

def _get_nc():
    global _NC
    if _NC is None:
        _NC = _build()
    return _NC


def run(inputs, trace=False):
    from concourse.bass_utils import run_bass_kernel_spmd
    nc = _get_nc()
    blob = _pack(inputs)
    parts = {
        "inbufD": np.ascontiguousarray(blob[:, 0:C_DMA_D]),
        "inbufA": np.ascontiguousarray(blob[:, C_DMA_D:C_DMA_A]),
        "inbufB": np.ascontiguousarray(blob[:, C_DMA_A:C_DMA_B]),
        "inbufC": np.ascontiguousarray(blob[:, C_DMA_B:C_COLS]),
        "inbufE": _pack_aug(inputs),
    }
    in_maps = [dict(parts) for _ in range(8)]
    res = run_bass_kernel_spmd(nc, in_maps, list(range(8)), trace=trace)
    out = np.asarray(res.results[0]["out"], dtype=np.float32).reshape(1, K1 * 20)
    return out, res


def kernel(**inputs) -> np.ndarray:
    out, _ = run(inputs)
    return out


# revision 98
# speedup vs baseline: 1.2481x; 1.0315x over previous
"""Trainium2 Bass kernel for nn_Brain_connectomic_graph (GNN message passing).

Single tiny graph (N=100 nodes, E=2000 edges); whole network as dense linear
algebra on ONE NeuronCore, replicated across 8 cores (data-parallel lanes,
batch=1 per the sharding hint); core 0's output is returned.

v3 design (latency-focused):
  - Adjacency densification done on the HOST as pure data placement: edges
    scattered into K=3 duplicate-slab grids (a duplicate (src,dst) pair goes
    to the next slab; no host arithmetic). Device sums slabs with 2 adds.
  - No unweighted grid: A1 (counts) comes from binarizing the weighted grid
    on GpSimd (all edge weights are nonzero).
  - No grid diagonal: the GCN +1 self-loop degree enters via the Sqrt
    activation's free bias; the +I adjacency term via one add with the
    on-device identity.
  - Degrees come from the dst-major grid via free-axis reduces (V only).
  - GCN layers alternate node-major/feature-major layouts -> NO transposes
    between layers; hemisphere selection via host-masked X^T stationaries
    (layer 1) and a 2-op DVE select (layer 2).
  - Layer biases enter as EXTRA CONTRACTION ROWS: stationaries/movings are
    augmented to k=101/102 with [bias rows | hemisphere masks], so bias
    needs no separate matmul or vector op anywhere.
  - dis sandwich built once per adjacency (shared by both layers).
  - ChebConv reassociated: s_raw = h2@(Wc0-Wc2) + G@(h2@Wc1) + 2G@(G@(h2@Wc2))
    with G the sandwiched pooled adjacency -- no Tx transposes.
  - Pooled-degree rsqrt via integer one-hot lookup against a host 1/sqrt(k)
    table (2 DVE ops, no Scalar Sqrt mid-kernel).
  - Scalar ACT tables: Sqrt set prewarmed during DMA; Exp/Tanh set loaded
    right after the single early Sqrt -- no stalls later.
"""

import numpy as np

N = 100
E = 2000
K1 = 50
KSLOT = 3          # duplicate-edge slabs (max multiplicity in data is 3)
GC = KSLOT * 100   # grid columns

# ---- inbuf column layout (f32 blob [128, C]) --------------------------------
_off = 0
def _nxt(w):
    global _off
    o = _off
    _off += w
    return o

# DMA group D (first: gates the degree/dis chain)
O_GWD  = _nxt(GC)     # [100,3,100] GWd[d, k, s] = ew(s->d), no diag
O_MBD  = _nxt(100)    # [100,100] same-hemisphere block mask
C_DMA_D = _off
# DMA group A
O_GW   = _nxt(GC)     # [100,3,100] GW[s, k, d] = ew(s->d), no diag
C_DMA_A = _off
# DMA group B: first-matmul operands
O_XTL  = _nxt(100)    # [100,100] x^T with cols (nodes) >=50 zeroed
O_XTR  = _nxt(100)    # [100,100] x^T with cols (nodes) <50 zeroed
O_W1   = _nxt(128)    # [100,128] [Wl1 | Wr1]
C_DMA_B = _off
# DMA group C: small weights/tables
O_MKL  = _nxt(1)      # [100,1] 1.0 for p<50
O_MKR  = _nxt(1)      # [100,1] 1.0 for 50<=p<100
O_BREL = _nxt(1)      # [128,1] brel broadcast
O_W2   = _nxt(40)     # [64,40] [Wl2|Wr2]
O_RSQ  = _nxt(64)     # [50,64] 1/sqrt(k) lookup rows
O_WG   = _nxt(20)     # [20,20] Wg1
O_WC0  = _nxt(20)     # [20,20] Wc0
O_WCC  = _nxt(40)     # [20,40] [Wc1 | Wc2]
O_WRR  = _nxt(2)      # [20,2]  [Wrel | Wroot]
C_COLS = _off
NRSQ = 64

# AUG mega-tile column layout: all bias-augmented stationaries/movings live in
# one [128, CAUG_COLS] tile so a single DMA fills every aug row (rows 100:102)
A_Y1, A_Y2, A_YG, A_ACT, A_AGT, A_WC0, A_H2T = 0, 64, 84, 104, 204, 304, 324
CAUG_COLS = 424


def _split_multiwaits(bir: dict) -> dict:
    """This container's walrus accepts only ONE sync-wait per instruction.
    Insert single-wait NoOps (same engine, just before) for the extras."""
    for f in bir.get("functions", []):
        for bb in f.get("blocks", []):
            out = []
            for ins in bb.get("instructions", []):
                si = ins.get("sync_info")
                waits = (si or {}).get("on_wait") or []
                if len(waits) > 1:
                    for i, w in enumerate(waits[:-1]):
                        out.append({
                            "debug": ins.get("debug", 0),
                            "engine": ins["engine"],
                            "ins": [], "outs": [],
                            "name": f"{ins['name']}-w{i}",
                            "opcode": "NoOp",
                            "sync_info": {"on_wait": [w], "on_update": []},
                        })
                    si["on_wait"] = [waits[-1]]
                out.append(ins)
            bb["instructions"] = out
    return bir


def _build():
    import concourse.bass as bass
    import concourse.mybir as mybir
    import concourse.tile as tile

    f32 = mybir.dt.float32
    Alu = mybir.AluOpType
    Act = mybir.ActivationFunctionType
    AxX = mybir.AxisListType.X

    nc = bass.Bass("TRN2")
    in_d = nc.dram_tensor("inbufD", [128, C_DMA_D], f32, kind="ExternalInput")
    in_a = nc.dram_tensor("inbufA", [128, C_DMA_A - C_DMA_D], f32, kind="ExternalInput")
    in_b = nc.dram_tensor("inbufB", [128, C_DMA_B - C_DMA_A], f32, kind="ExternalInput")
    in_c = nc.dram_tensor("inbufC", [128, C_COLS - C_DMA_B], f32, kind="ExternalInput")
    in_e = nc.dram_tensor("inbufE", [2, CAUG_COLS], f32, kind="ExternalInput")
    out_d = nc.dram_tensor("out", [K1, 20], f32, kind="ExternalOutput")

    with tile.TileContext(nc) as tc:
        with (
            tc.tile_pool(name="sb", bufs=1) as sb,
            tc.tile_pool(name="ps", bufs=1, space="PSUM") as ps,
        ):
            ibD = sb.tile([128, C_DMA_D], f32, tag="ibD", name="ibD")
            nc.sync.dma_start(out=ibD[:, :], in_=in_d.ap())
            ibA = sb.tile([128, C_DMA_A - C_DMA_D], f32, tag="ibA", name="ibA")
            nc.sync.dma_start(out=ibA[:, :], in_=in_a.ap())
            ibB = sb.tile([128, C_DMA_B - C_DMA_A], f32, tag="ibB", name="ibB")
            nc.sync.dma_start(out=ibB[:, :], in_=in_b.ap())
            ibC = sb.tile([128, C_COLS - C_DMA_B], f32, tag="ibC", name="ibC")
            nc.sync.dma_start(out=ibC[:, :], in_=in_c.ap())

            GWD  = ibD[0:100, O_GWD:O_GWD + GC]
            MBD  = ibD[0:100, O_MBD:O_MBD + 100]
            GW   = ibA[0:100, 0:GC]
            XTL  = ibB[0:100, O_XTL - C_DMA_A:O_XTL - C_DMA_A + 100]
            XTR  = ibB[0:100, O_XTR - C_DMA_A:O_XTR - C_DMA_A + 100]
            W1   = ibB[0:100, O_W1 - C_DMA_A:O_W1 - C_DMA_A + 128]
            def icl(off, w, p0=0, p1=128):
                return ibC[p0:p1, off - C_DMA_B:off - C_DMA_B + w]
            MKL  = icl(O_MKL, 1, 0, 100)
            MKR  = icl(O_MKR, 1, 0, 100)
            BREL = icl(O_BREL, 1)
            W2   = icl(O_W2, 40, 0, 64)
            RSQ  = icl(O_RSQ, NRSQ, 0, 50)
            WG   = icl(O_WG, 20, 0, 20)
            WC0  = icl(O_WC0, 20, 0, 20)
            WCC  = icl(O_WCC, 40, 0, 20)
            WRR2 = icl(O_WRR, 2, 0, 20)

            V = nc.vector
            S = nc.scalar
            P = nc.gpsimd
            T = nc.tensor
            mm = lambda shape, name: ps.tile(shape, f32, tag="mm", name=name, bufs=7)

            # AUG mega-tile: all bias-augmented operands; zeroed once, aug rows
            # (100:102) filled by ONE small DMA
            AUG = sb.tile([128, CAUG_COLS], f32, tag="AUG", name="AUG")
            V.memset(AUG, 0.0)
            nc.sync.dma_start(out=AUG[100:102, :], in_=in_e.ap())
            y1aug = AUG[0:102, A_Y1:A_Y1 + 64]
            y2aug = AUG[0:102, A_Y2:A_Y2 + 20]
            ygaug = AUG[0:101, A_YG:A_YG + 20]
            acts_aug = AUG[0:102, A_ACT:A_ACT + 100]
            agts_aug = AUG[0:101, A_AGT:A_AGT + 100]
            wc0paug = AUG[0:102, A_WC0:A_WC0 + 20]
            h2Taug = AUG[0:102, A_H2T:A_H2T + 100]
            act_s = AUG[0:100, A_ACT:A_ACT + 100]
            agt_s = AUG[0:100, A_AGT:A_AGT + 100]
            h2T = AUG[0:20, A_H2T:A_H2T + 100]
            wc0p = AUG[0:20, A_WC0:A_WC0 + 20]

            # ---- prologue: ACT sqrt-set prewarm + PE warmup (HAM ramp) ------
            scr = sb.tile([1, 1], f32, tag="scr", name="scr")
            V.memset(scr, 0.0)
            S.activation(out=scr, in_=scr, func=Act.Sqrt)
            wmt = sb.tile([128, 100], f32, tag="wmt", name="wmt")
            V.memset(wmt, 1.0)
            warm = ps.tile([100, 200], f32, tag="warm", name="warm", bufs=1)
            wm_b = wmt.unsqueeze(1).broadcast_to([128, 2, 100])
            for _ in range(4):
                T.matmul(warm, wmt, wm_b)

            # ---- on-device constants (GpSimd, runs during the DMAs) ---------
            iota_i = sb.tile([128, 100], mybir.dt.int32, tag="iota_i", name="iota_i")
            P.iota(iota_i, pattern=[[1, 100]], base=0, channel_multiplier=0)
            iota_t = sb.tile([128, 100], f32, tag="iota_t", name="iota_t")
            P.tensor_copy(out=iota_t, in_=iota_i)
            i100_t = sb.tile([100, 100], f32, tag="i100_t", name="i100_t")
            P.memset(i100_t, 0.0)
            P.affine_select(out=i100_t, in_=i100_t, compare_op=Alu.not_equal,
                            fill=1.0, base=0, pattern=[[-1, 100]], channel_multiplier=1)
            tril_t = sb.tile([100, 100], f32, tag="tril_t", name="tril_t")
            P.memset(tril_t, 1.0)
            P.affine_select(out=tril_t, in_=tril_t, compare_op=Alu.is_gt,
                            fill=0.0, base=0, pattern=[[-1, 100]], channel_multiplier=1)
            triu_t = sb.tile([100, 100], f32, tag="triu_t", name="triu_t")
            P.memset(triu_t, 1.0)
            P.affine_select(out=triu_t, in_=triu_t, compare_op=Alu.is_gt,
                            fill=0.0, base=0, pattern=[[1, 100]], channel_multiplier=-1)
            ones_t = sb.tile([1, 100], f32, tag="ones_t", name="ones_t")
            P.memset(ones_t, 1.0)
            ONESR = ones_t[0:1, :]
            onesq = sb.tile([100, 100], f32, tag="onesq", name="onesq")
            P.memset(onesq, 1.0)
            I100 = i100_t[:, :]
            I20 = i100_t[0:20, 0:20]
            I50 = i100_t[0:50, 0:50]
            IO50 = iota_t[0:100, 0:50]
            IO64 = iota_t[0:50, 0:NRSQ]
            TRIL = tril_t[:, :]
            TRIU = triu_t[:, :]

            # ---- degrees straight off the dst-major grid --------------------
            # GWd's columns are host-permuted so same-hemisphere sources sit in
            # cols 0:50 of every slab -> deg_c is a plain subrange reduce
            dd = sb.tile([100, 2], f32, tag="dd", name="dd")
            gwd3 = GWD.rearrange("p (c j) -> p c j", c=KSLOT)
            V.tensor_reduce(out=dd[:, 0:1], in_=gwd3[:, :, 0:50],
                            axis=mybir.AxisListType.XY, op=Alu.add)
            V.tensor_reduce(out=dd[:, 1:2], in_=gwd3, axis=mybir.AxisListType.XY, op=Alu.add)
            # dis = 1/sqrt(deg+1): +1 self-loop via Sqrt's free bias
            sq2 = sb.tile([100, 2], f32, tag="sq2", name="sq2")
            S.activation(out=sq2, in_=dd, func=Act.Sqrt, bias=1.0)
            # switch Scalar ACT table to the Exp/Tanh set right after the last
            # Sqrt (input dep on sq2 pins the order; the load then overlaps
            # the GCN layers instead of stalling the tail)
            S.activation(out=scr, in_=sq2[0:1, 0:1], func=Act.Tanh)
            rdis = sb.tile([100, 2], f32, tag="rdis", name="rdis")
            V.reciprocal(out=rdis, in_=sq2)
            rdis_c = rdis[:, 0:1]
            rdis_g = rdis[:, 1:2]

            # diag(dis_c) on GpSimd BEFORE the slab adds: it gates the layer-1
            # sandwich matmul while the adds have more slack
            dgc = sb.tile([100, 100], f32, tag="dgc", name="dgc")
            P.affine_select(out=dgc, in_=rdis_c.broadcast_to([100, 100]),
                            compare_op=Alu.is_equal, fill=0.0, base=0,
                            pattern=[[-1, 100]], channel_multiplier=1)

            # ---- adjacency slab sums (GpSimd; DVE stays on the deg chain) ---
            agtmp = sb.tile([100, 100], f32, tag="agtmp", name="agtmp")
            P.tensor_tensor(out=agtmp, in0=GW[:, 0:100], in1=GW[:, 100:200], op=Alu.add)
            agts = sb.tile([100, 100], f32, tag="agts", name="agts")
            P.tensor_tensor(out=agts, in0=agtmp, in1=GW[:, 200:300], op=Alu.add)
            agt = sb.tile([100, 100], f32, tag="agt", name="agt")
            P.tensor_tensor(out=agt, in0=agts, in1=I100, op=Alu.add)
            act = sb.tile([100, 100], f32, tag="act", name="act")
            P.tensor_tensor(out=act, in0=agt, in1=MBD, op=Alu.mult)
            # ---- dis sandwich for both adjacencies --------------------------
            # row-replicate dis via ones @ diag(dis)
            dgg = sb.tile([100, 100], f32, tag="dgg", name="dgg")
            P.affine_select(out=dgg, in_=rdis_g.broadcast_to([100, 100]),
                            compare_op=Alu.is_equal, fill=0.0, base=0,
                            pattern=[[-1, 100]], channel_multiplier=1)
            drep_c = mm([100, 100], "drep_c")
            T.matmul(drep_c, onesq, dgc)
            V.scalar_tensor_tensor(out=act_s, in0=drep_c, scalar=rdis_c, in1=act,
                                   op0=Alu.mult, op1=Alu.mult)

            # Wc0' = Wc0 - Wc2 (early, off critical path)
            V.tensor_tensor(out=wc0p, in0=WC0, in1=WCC[:, 20:40], op=Alu.subtract)

            # unweighted counts (all ew > 0)
            b3 = sb.tile([100, GC], f32, tag="b3", name="b3")
            V.tensor_scalar(out=b3, in0=GW, scalar1=0.0, scalar2=None, op0=Alu.is_gt)
            a1tmp = sb.tile([100, 100], f32, tag="a1tmp", name="a1tmp")
            V.tensor_tensor(out=a1tmp, in0=b3[:, 0:100], in1=b3[:, 100:200], op=Alu.add)
            a1t = sb.tile([100, 100], f32, tag="a1t", name="a1t")
            V.tensor_tensor(out=a1t, in0=a1tmp, in1=b3[:, 200:300], op=Alu.add)

            # ---- layer 1 (out feature-major [64,100]) -----------------------
            xw_ps = mm([100, 64], "xw_ps")
            T.matmul(xw_ps, XTL, W1[:, 0:64], start=True, stop=False)
            T.matmul(xw_ps, XTR, W1[:, 64:128], start=False, stop=True)
            V.tensor_copy(out=AUG[0:100, A_Y1:A_Y1 + 64], in_=xw_ps)
            z1T = mm([64, 100], "z1T")
            T.matmul(z1T, y1aug, acts_aug)
            # global-layer sandwich off the critical path (first use ~5us out)
            drep_g = mm([100, 100], "drep_g")
            T.matmul(drep_g, onesq, dgg)
            V.scalar_tensor_tensor(out=agt_s, in0=drep_g, scalar=rdis_g, in1=agt,
                                   op0=Alu.mult, op1=Alu.mult)
            z1s = sb.tile([64, 100], f32, tag="z1s", name="z1s")
            V.tensor_copy(out=z1s, in_=z1T)
            h1t = sb.tile([64, 100], f32, tag="h1t", name="h1t")
            V.scalar_tensor_tensor(out=h1t, in0=z1s, scalar=0.01, in1=z1s,
                                   op0=Alu.mult, op1=Alu.max)

            # ---- layer 2 ----------------------------------------------------
            xw2 = mm([100, 40], "xw2")
            T.matmul(xw2, h1t, W2)
            y2r = sb.tile([100, 20], f32, tag="y2r", name="y2r")
            V.tensor_scalar_mul(y2r, xw2[:, 20:40], MKR)
            V.scalar_tensor_tensor(out=AUG[0:100, A_Y2:A_Y2 + 20], in0=xw2[:, 0:20],
                                   scalar=MKL, in1=y2r, op0=Alu.mult, op1=Alu.add)
            z2T = mm([20, 100], "z2T")
            T.matmul(z2T, y2aug, acts_aug)
            z2s = sb.tile([20, 100], f32, tag="z2s", name="z2s")
            V.tensor_copy(out=z2s, in_=z2T)
            h2at = sb.tile([20, 100], f32, tag="h2at", name="h2at")
            V.scalar_tensor_tensor(out=h2at, in0=z2s, scalar=0.01, in1=z2s,
                                   op0=Alu.mult, op1=Alu.max)

            # ---- global GCN layer ------------------------------------------
            xwg = mm([100, 20], "xwg")
            T.matmul(xwg, h2at, WG)
            V.tensor_copy(out=AUG[0:100, A_YG:A_YG + 20], in_=xwg)
            zgT = mm([20, 100], "zgT")
            T.matmul(zgT, ygaug, agts_aug)
            zgs = sb.tile([20, 100], f32, tag="zgs", name="zgs")
            V.tensor_copy(out=zgs, in_=zgT)
            V.scalar_tensor_tensor(out=h2T, in0=zgs, scalar=0.01, in1=zgs,
                                   op0=Alu.mult, op1=Alu.max)

            # ---- SAGPool score (critical: emitted before h2x/Cheb mms) ------
            h2x = sb.tile([100, 21], f32, tag="h2x", name="h2x")
            score = h2x[:, 20:21]
            hw_ps = mm([100, 2], "hw_ps")
            T.matmul(hw_ps, h2T, WRR2)
            hw = sb.tile([100, 2], f32, tag="hw", name="hw")
            V.tensor_copy(out=hw, in_=hw_ps)
            sc_ps = mm([100, 1], "sc_ps")
            T.matmul(sc_ps, a1t, hw[:, 0:1])
            V.tensor_tensor(out=score, in0=sc_ps, in1=hw[:, 1:2], op=Alu.add)

            # h2 node-major into h2x cols 0:20; Cheb products (need only h2T)
            h2x_p = mm([100, 20], "h2x_p")
            T.transpose(h2x_p, h2T, I20)
            V.tensor_copy(out=h2x[:, 0:20], in_=h2x_p)
            pp_ps = mm([100, 40], "pp_ps")
            T.matmul(pp_ps, h2T, WCC)
            pp = sb.tile([50, 40], f32, tag="pp", name="pp")
            V.tensor_copy(out=pp, in_=pp_ps[0:50, :])
            sraw_ps = mm([100, 20], "sraw_ps")
            T.matmul(sraw_ps, h2Taug, wc0paug, start=True, stop=False)

            # ---- rank / top-k ----------------------------------------------
            dgs = sb.tile([100, 100], f32, tag="dgs", name="dgs")
            P.affine_select(out=dgs, in_=score.broadcast_to([100, 100]),
                            compare_op=Alu.is_equal, fill=0.0, base=0,
                            pattern=[[-1, 100]], channel_multiplier=1)
            srep_ps = mm([100, 100], "srep_ps")
            T.matmul(srep_ps, onesq, dgs)
            t2 = sb.tile([100, 100], f32, tag="t2", name="t2")
            V.scalar_tensor_tensor(out=t2, in0=srep_ps, scalar=score, in1=TRIL,
                                   op0=Alu.is_equal, op1=Alu.mult)
            csum = sb.tile([100, 100], f32, tag="csum", name="csum")
            rank = sb.tile([100, 1], f32, tag="rank", name="rank")
            V.scalar_tensor_tensor(out=csum, in0=srep_ps, scalar=score, in1=t2,
                                   op0=Alu.is_gt, op1=Alu.add, accum_out=rank)
            kept = sb.tile([100, 1], f32, tag="kept", name="kept")
            V.tensor_scalar(out=kept, in0=rank, scalar1=49.5, scalar2=None, op0=Alu.is_lt)
            pit = sb.tile([100, 50], f32, tag="pit", name="pit")
            V.tensor_scalar(out=pit, in0=IO50, scalar1=rank, scalar2=None, op0=Alu.is_equal)

            # ---- pooled rows + gather matrix -------------------------------
            w_ps = mm([100, 1], "w_ps")
            T.matmul(w_ps, a1t, kept)
            w_sb = sb.tile([100, 1], f32, tag="w_sb", name="w_sb")
            V.tensor_copy(out=w_sb, in_=w_ps)
            m1 = mm([100, 50], "m1")
            T.matmul(m1, a1t, pit)
            m1s = sb.tile([100, 50], f32, tag="m1s", name="m1s")
            V.tensor_copy(out=m1s, in_=m1)
            degc_p = mm([50, 1], "degc_p")
            T.matmul(degc_p, pit, w_sb)
            atilt_p = mm([50, 50], "atilt_p")
            T.matmul(atilt_p, m1s, pit)
            p1 = mm([50, 21], "p1")
            T.matmul(p1, pit, h2x)
            th = sb.tile([50, 1], f32, tag="th", name="th")
            S.activation(out=th, in_=p1[:, 20:21], func=Act.Tanh, bias=BREL[0:50, :], scale=1.0)
            p1s = sb.tile([50, 20], f32, tag="p1s", name="p1s")
            V.tensor_copy(out=p1s, in_=p1[:, 0:20])
            srank_p = mm([100, 1], "srank_p")
            T.matmul(srank_p, TRIU, kept)
            gat = sb.tile([100, 50], f32, tag="gat", name="gat")
            V.scalar_tensor_tensor(out=gat, in0=IO50, scalar=srank_p,
                                   in1=kept.broadcast_to([100, 50]),
                                   op0=Alu.is_equal, op1=Alu.mult)

            # pooled-degree rsqrt via integer one-hot lookup (no Scalar Sqrt)
            ohscr = sb.tile([50, NRSQ], f32, tag="ohscr", name="ohscr")
            V.scalar_tensor_tensor(out=ohscr, in0=IO64, scalar=degc_p, in1=RSQ,
                                   op0=Alu.is_equal, op1=Alu.mult)
            disch = sb.tile([50, 1], f32, tag="disch", name="disch")
            V.tensor_reduce(out=disch, in_=ohscr, axis=AxX, op=Alu.add)
            ndisch = sb.tile([50, 1], f32, tag="ndisch", name="ndisch")
            V.tensor_scalar_mul(ndisch, disch, -1.0)
            dgd = sb.tile([50, 50], f32, tag="dgd", name="dgd")
            P.affine_select(out=dgd, in_=disch.broadcast_to([50, 50]),
                            compare_op=Alu.is_equal, fill=0.0, base=0,
                            pattern=[[-1, 50]], channel_multiplier=1)
            drepd = mm([50, 50], "drepd")
            T.matmul(drepd, onesq[0:50, 0:50], dgd)
            gsx1 = sb.tile([50, 50], f32, tag="gsx1", name="gsx1")
            V.tensor_scalar_mul(gsx1, atilt_p, ndisch)
            gsx = sb.tile([50, 100], f32, tag="gsx", name="gsx")
            V.memset(gsx, 0.0)
            V.tensor_tensor(out=gsx[:, 0:50], in0=drepd, in1=gsx1, op=Alu.mult)

            # ---- Cheb accumulation into sraw -------------------------------
            T.matmul(sraw_ps, gsx, pp[:, 0:20], start=False, stop=False)
            q2_ps = mm([100, 20], "q2_ps")
            T.matmul(q2_ps, gsx, pp[:, 20:40])
            q2x2 = sb.tile([50, 20], f32, tag="q2x2", name="q2x2")
            V.tensor_scalar_mul(q2x2, q2_ps[0:50, :], 2.0)
            T.matmul(sraw_ps, gsx, q2x2, start=False, stop=True)

            # ---- double softmax (normalizations folded) --------------------
            ex1 = sb.tile([100, 20], f32, tag="ex1", name="ex1")
            sum1 = sb.tile([100, 1], f32, tag="sum1", name="sum1")
            S.activation(out=ex1, in_=sraw_ps, func=Act.Exp, accum_out=sum1)
            rc1 = sb.tile([100, 1], f32, tag="rc1", name="rc1")
            V.reciprocal(out=rc1, in_=sum1)
            exr = sb.tile([100, 20], f32, tag="exr", name="exr")
            V.tensor_scalar_mul(exr, ex1, rc1)
            ex2 = sb.tile([100, 20], f32, tag="ex2", name="ex2")
            sum2 = sb.tile([100, 1], f32, tag="sum2", name="sum2")
            S.activation(out=ex2, in_=ex1, func=Act.Exp, scale=rc1, accum_out=sum2)
            rc2 = sb.tile([100, 1], f32, tag="rc2", name="rc2")
            V.reciprocal(out=rc2, in_=sum2)
            s2 = sb.tile([100, 20], f32, tag="s2", name="s2")
            V.tensor_scalar_mul(s2, ex2, rc2)

            # ---- diff-pool tail --------------------------------------------
            # M = gat^T @ ass (runs while softmax-2 is still on Scalar)
            m_ps = mm([50, 20], "m_ps")
            T.matmul(m_ps, gat, exr)
            m_sb = sb.tile([50, 20], f32, tag="m_sb", name="m_sb")
            V.tensor_copy(out=m_sb, in_=m_ps)
            mt_ps = mm([20, 50], "mt_ps")
            T.transpose(mt_ps, m_sb, I50)
            mt = sb.tile([20, 50], f32, tag="mt", name="mt")
            V.tensor_copy(out=mt, in_=mt_ps)
            hc_ps = mm([20, 20], "hc_ps")
            T.matmul(hc_ps, s2, h2x[:, 0:20])
            hc = sb.tile([20, 20], f32, tag="hc", name="hc")
            V.tensor_copy(out=hc, in_=hc_ps)
            g_p = mm([50, 20], "g_p")
            T.matmul(g_p, mt, hc)
            outv = sb.tile([50, 20], f32, tag="outv", name="outv")
            V.scalar_tensor_tensor(out=outv, in0=p1s, scalar=th, in1=g_p,
                                   op0=Alu.mult, op1=Alu.add)
            nc.sync.dma_start(out=out_d.ap(), in_=outv)

    # walrus single-wait workaround
    orig = nc.to_json_bytes
    def patched(*a, **k):
        import json as _json
        return _json.dumps(_split_multiwaits(_json.loads(orig(*a, **k)))).encode()
    nc.to_json_bytes = patched
    return nc


def _pack(inputs) -> np.ndarray:
    f = lambda k: np.asarray(inputs[k], dtype=np.float32)
    blob = np.zeros((128, C_COLS), dtype=np.float32)

    ei = np.asarray(inputs["edge_index"]).astype(np.int64)
    src, dst = ei[0], ei[1]
    ew = f("edge_attr")
    assert (ew > 0).all(), "zero edge weight breaks grid binarization"
    # scatter edges into duplicate slabs (pure placement; no arithmetic)
    slot = {}
    gwd = np.zeros((100, KSLOT, 100), np.float32)
    gw = np.zeros((100, KSLOT, 100), np.float32)
    for e in range(E):
        s, d = int(src[e]), int(dst[e])
        k = slot.get((s, d), 0)
        slot[(s, d)] = k + 1
        assert k < KSLOT, "duplicate-edge multiplicity exceeds KSLOT"
        # dst-major grid: per-row column rotation puts same-hemisphere
        # sources in cols 0:50 (pure placement)
        sc = s if d < 50 else (s + 50) % 100
        gwd[d, k, sc] = ew[e]
        gw[s, k, d] = ew[e]
    blob[0:100, O_GWD:O_GWD + GC] = gwd.reshape(100, GC)
    blob[0:100, O_GW:O_GW + GC] = gw.reshape(100, GC)

    half = np.arange(100) < 50
    blob[0:100, O_MBD:O_MBD + 100] = (half[:, None] == half[None, :]).astype(np.float32)

    x = f("x")
    xt = x.T.copy()
    xtl = xt.copy(); xtl[:, 50:] = 0.0
    xtr = xt.copy(); xtr[:, :50] = 0.0
    blob[0:100, O_XTL:O_XTL + 100] = xtl
    blob[0:100, O_XTR:O_XTR + 100] = xtr
    blob[0:100, O_W1:O_W1 + 64] = f("Wl1")
    blob[0:100, O_W1 + 64:O_W1 + 128] = f("Wr1")

    blob[0:50, O_MKL] = 1.0
    blob[50:100, O_MKR] = 1.0
    blob[:, O_BREL] = f("brel")[0]
    blob[0:64, O_W2:O_W2 + 20] = f("Wl2")
    blob[0:64, O_W2 + 20:O_W2 + 40] = f("Wr2")
    # 1/sqrt(k) lookup rows (constants; row-replicated for the free-dim dot)
    ks = np.arange(NRSQ, dtype=np.float32)
    rsq = np.zeros(NRSQ, np.float32)
    rsq[1:] = 1.0 / np.sqrt(ks[1:])
    blob[0:50, O_RSQ:O_RSQ + NRSQ] = rsq[None, :]
    blob[0:20, O_WG:O_WG + 20] = f("Wg1")
    blob[0:20, O_WC0:O_WC0 + 20] = f("Wc0")
    blob[0:20, O_WCC:O_WCC + 20] = f("Wc1")
    blob[0:20, O_WCC + 20:O_WCC + 40] = f("Wc2")
    blob[0:20, O_WRR] = f("Wrel")[:, 0]
    blob[0:20, O_WRR + 1] = f("Wroot")[:, 0]
    return blob


def _pack_aug(inputs) -> np.ndarray:
    f = lambda k: np.asarray(inputs[k], dtype=np.float32)
    aug = np.zeros((2, CAUG_COLS), np.float32)
    half = (np.arange(100) < 50).astype(np.float32)
    aug[0, A_Y1:A_Y1 + 64] = f("bl1")
    aug[1, A_Y1:A_Y1 + 64] = f("br1")
    aug[0, A_Y2:A_Y2 + 20] = f("bl2")
    aug[1, A_Y2:A_Y2 + 20] = f("br2")
    aug[0, A_YG:A_YG + 20] = f("bg1")
    aug[0, A_ACT:A_ACT + 100] = half
    aug[1, A_ACT:A_ACT + 100] = 1.0 - half
    aug[0, A_AGT:A_AGT + 100] = 1.0
    aug[0, A_WC0:A_WC0 + 20] = f("bc")
    aug[0, A_H2T:A_H2T + 100] = 1.0
    return aug


_NC = None

def _get_nc():
    global _NC
    if _NC is None:
        _NC = _build()
    return _NC


def run(inputs, trace=False):
    from concourse.bass_utils import run_bass_kernel_spmd
    nc = _get_nc()
    blob = _pack(inputs)
    parts = {
        "inbufD": np.ascontiguousarray(blob[:, 0:C_DMA_D]),
        "inbufA": np.ascontiguousarray(blob[:, C_DMA_D:C_DMA_A]),
        "inbufB": np.ascontiguousarray(blob[:, C_DMA_A:C_DMA_B]),
        "inbufC": np.ascontiguousarray(blob[:, C_DMA_B:C_COLS]),
        "inbufE": _pack_aug(inputs),
    }
    in_maps = [dict(parts) for _ in range(8)]
    res = run_bass_kernel_spmd(nc, in_maps, list(range(8)), trace=trace)
    out = np.asarray(res.results[0]["out"], dtype=np.float32).reshape(1, K1 * 20)
    return out, res


def kernel(**inputs) -> np.ndarray:
    out, _ = run(inputs)
    return out
